# revision 1
# baseline (speedup 1.0000x reference)
"""Single-head MHA (QKV proj + softmax attention) on 8 Trainium2 cores.

Problem: x[8, 4096, 256] f32; per-batch attention with per-head emb 256.
Sharding: data-parallel — one batch element per NeuronCore (8 cores).

Per-core algorithm (S=4096, E=256, P=128 partitions):
  - cast x to bf16, PE-transpose -> xT[d, s]
  - QT[e, s] = WqT.T @ xT (+bq), KT likewise, V[s, e] = xT.T @ WvT  (bf16)
  - per q-block of 1024 columns, loop k-tiles of 128 rows:
      S^T[k, q] = KT_slice.T @ QT   (4 matmuls of N=512, fp32 PSUM)
      E[k, q]   = exp(S^T / 16)     (one ScalarE op, scale fused, bf16 out)
      out[q, e]+= E_chunk.T @ V     (8 matmuls of N=256: E q-chunks are the
                                     stationary operand, so the output lands
                                     in [q, e] layout — no transposes needed)
      denom    += E                 (VectorE, fp32, two interleaved chains)
    softmax denominators: 8 tiny N=1 matmuls dall_chunk.T @ ones -> [q, 1]
    columns in PSUM; reciprocal; out = out_ps * recip + bv fused in one
    scalar_tensor_tensor per 128-row tile (softmax rows sum to 1, so
    attn @ (V + bv) = attn @ V + bv).

No running-max subtraction: scores/16 ~ N(0,1); max observed ~10.5, exp
stays well inside fp32/bf16 range.
"""

from contextlib import ExitStack

import numpy as np

import concourse.bass as bass
import concourse.tile as tile
from concourse import bacc
from concourse import mybir
from concourse import bass_utils
from concourse.masks import make_identity

P = 128          # partitions
EMB = 256        # head dim
S = 4096         # sequence length
B = 8            # batch == number of cores
QB = 1024        # q-block (free dim of S^T / E tiles; matmuls split into 512s)
MMN = 512        # max matmul free dim (one PSUM bank of fp32)

F32 = mybir.dt.float32
BF16 = mybir.dt.bfloat16
AF = mybir.ActivationFunctionType


def _build(nc: bass.Bass, s_len: int = S) -> None:
    """Emit the per-core program into `nc` (SPMD: same program all cores)."""
    x = nc.dram_tensor("x", (s_len, EMB), F32, kind="ExternalInput").ap()
    Wq = nc.dram_tensor("Wq", (EMB, EMB), F32, kind="ExternalInput").ap()
    bq = nc.dram_tensor("bq", (EMB,), F32, kind="ExternalInput").ap()
    Wk = nc.dram_tensor("Wk", (EMB, EMB), F32, kind="ExternalInput").ap()
    bk = nc.dram_tensor("bk", (EMB,), F32, kind="ExternalInput").ap()
    Wv = nc.dram_tensor("Wv", (EMB, EMB), F32, kind="ExternalInput").ap()
    bv = nc.dram_tensor("bv", (EMB,), F32, kind="ExternalInput").ap()
    out = nc.dram_tensor("out", (s_len, EMB), F32, kind="ExternalOutput").ap()

    n_st = s_len // P      # 128-row tiles of the sequence
    n_qb = s_len // QB     # q-blocks
    n_kt = s_len // P      # k-tiles
    n_qt = QB // P         # 128-row q-tiles per q-block
    scale = float(EMB) ** -0.5

    with tile.TileContext(nc) as tc, ExitStack() as ctx:
        consts = ctx.enter_context(tc.tile_pool(name="consts", bufs=1))
        persist = ctx.enter_context(tc.tile_pool(name="persist", bufs=1))
        stage = ctx.enter_context(tc.tile_pool(name="stage", bufs=6))
        work = ctx.enter_context(tc.tile_pool(name="work", bufs=5))
        outp = ctx.enter_context(tc.tile_pool(name="outp", bufs=6))
        # PSUM budget is exactly 8 banks: "mm" 2 slots x [128,1024]f32 (2
        # banks each) + "po" 2 slots x [128,1024]f32. Transpose outputs and
        # projection tiles share the "mm" slots.
        ps_mm = ctx.enter_context(tc.tile_pool(name="ps_mm", bufs=2, space="PSUM"))
        ps_acc = ctx.enter_context(tc.tile_pool(name="ps_acc", bufs=1, space="PSUM"))
        ps_tp = ps_mm

        # identity for PE transposes (all transposes run in f32; the cast to
        # bf16 happens on the PSUM->SBUF copy)
        idf = consts.tile([P, P], F32)
        make_identity(nc, idf)
        ones_f = consts.tile([P, 1], F32)
        nc.vector.memset(ones_f, 1.0)
        idb = consts.tile([P, P], BF16)
        nc.vector.tensor_copy(idb, idf)

        # biases: bq/bk as per-partition columns (e on partitions),
        # bv broadcast across partitions (added at the very end).
        bq_sb = consts.tile([P, 2], F32)
        nc.gpsimd.dma_start(bq_sb, bq.rearrange("(t p) -> p t", p=P))
        bk_sb = consts.tile([P, 2], F32)
        nc.gpsimd.dma_start(bk_sb, bk.rearrange("(t p) -> p t", p=P))
        bv_bc = consts.tile([P, EMB], F32)
        nc.gpsimd.dma_start(
            bv_bc,
            bass.AP(tensor=bv.tensor, offset=bv.offset, ap=[[0, P], list(bv.ap[0])]),
        )

        # ---- weights: load W[e,d], PE-transpose -> WT[d,e] (cast on copy) ----
        WT = {}
        for wname, wap in (("q", Wq), ("k", Wk), ("v", Wv)):
            wt0 = persist.tile([P, EMB], BF16, name=f"wt_{wname}_0")
            wt1 = persist.tile([P, EMB], BF16, name=f"wt_{wname}_1")
            WT[wname] = (wt0, wt1)
            for et in range(2):
                wst = stage.tile([P, EMB], F32, tag="wst")
                nc.sync.dma_start(wst, wap[et * P:(et + 1) * P, :])
                wbf = stage.tile([P, EMB], BF16, tag="wbf")
                nc.vector.tensor_copy(wbf, wst)
                for dc in range(2):
                    tp = ps_tp.tile([P, P], BF16, tag="mm")
                    nc.tensor.transpose(tp, wbf[:, dc * P:(dc + 1) * P], idb)
                    nc.scalar.copy(WT[wname][dc][:, et * P:(et + 1) * P], tp)

        # ---- x: load, PE-transpose -> xT[dc][d, s] (cast on copy) ----
        xT = [persist.tile([P, s_len], BF16, name=f"xT{dc}") for dc in range(2)]
        for st_i in range(n_st):
            xst = stage.tile([P, EMB], F32, tag="xst")
            nc.sync.dma_start(xst, x[st_i * P:(st_i + 1) * P, :])
            xbf = stage.tile([P, EMB], BF16, tag="xbf")
            nc.gpsimd.tensor_copy(xbf, xst)
            for dc in range(2):
                tp = ps_tp.tile([P, P], BF16, tag="mm")
                nc.tensor.transpose(tp, xbf[:, dc * P:(dc + 1) * P], idb)
                nc.scalar.copy(xT[dc][:, st_i * P:(st_i + 1) * P], tp)

        # ---- projections: QT/KT[e, s] (e on partitions), V[s, e] ----
        QT = [persist.tile([P, s_len], BF16, name=f"QT{t}") for t in range(2)]
        KT = [persist.tile([P, s_len], BF16, name=f"KT{t}") for t in range(2)]
        Vb = persist.tile([P, n_st, EMB], BF16, name="Vb")
        # K first, then V, then Q: the attention loop needs all of K/V but
        # only the first q-block of Q, so this order unblocks it earliest.
        for t in range(2):
            for sb in range(s_len // MMN):
                ssl = slice(sb * MMN, (sb + 1) * MMN)
                kps = ps_mm.tile([P, MMN], F32, tag="mm")
                nc.tensor.matmul(kps, WT["k"][0][:, t * P:(t + 1) * P],
                                 xT[0][:, ssl], start=True, stop=False)
                nc.tensor.matmul(kps, WT["k"][1][:, t * P:(t + 1) * P],
                                 xT[1][:, ssl], start=False, stop=True)
                nc.scalar.activation(KT[t][:, ssl], kps, AF.Identity,
                                     bias=bk_sb[:, t:t + 1], scale=1.0)
        for st_i in range(n_st):
            vps = ps_mm.tile([P, EMB], F32, tag="mm")
            nc.tensor.matmul(vps, xT[0][:, st_i * P:(st_i + 1) * P], WT["v"][0],
                             start=True, stop=False)
            nc.tensor.matmul(vps, xT[1][:, st_i * P:(st_i + 1) * P], WT["v"][1],
                             start=False, stop=True)
            nc.vector.tensor_copy(Vb[:, st_i, :], vps)
        for sb in range(s_len // MMN):
            ssl = slice(sb * MMN, (sb + 1) * MMN)
            for t in range(2):
                qps = ps_mm.tile([P, MMN], F32, tag="mm")
                nc.tensor.matmul(qps, WT["q"][0][:, t * P:(t + 1) * P],
                                 xT[0][:, ssl], start=True, stop=False)
                nc.tensor.matmul(qps, WT["q"][1][:, t * P:(t + 1) * P],
                                 xT[1][:, ssl], start=False, stop=True)
                nc.scalar.activation(QT[t][:, ssl], qps, AF.Identity,
                                     bias=bq_sb[:, t:t + 1], scale=1.0)

        # ---- attention ----
        ebf_chunks = n_qt  # 128-wide q-chunks of the exp tile
        for qb_i in range(n_qb):
            q0b = qb_i * QB
            # out accumulator in [q, e] layout: one PSUM tensor, 4 banks.
            # PV uses the exp tile E^T[k, q] as the *stationary* operand
            # (128-column q-chunks) and V[k, e] as the moving operand, so the
            # attention output lands directly in [q, e] — no transposes.
            out_ps = ps_acc.tile([P, n_qt, EMB], F32, tag="po",
                                 name=f"out_ps_{qb_i}")
            # two interleaved softmax-denominator accumulators, one summed on
            # the vector engine and one on gpsimd, so neither chain bottlenecks
            dacc = [work.tile([P, QB], F32, tag=f"dacc{i}", name=f"dacc{i}_{qb_i}")
                    for i in range(2)]
            elist = []
            for kt_i in range(n_kt):
                ksl = slice(kt_i * P, (kt_i + 1) * P)
                stp = ps_mm.tile([P, QB], F32, tag="mm")
                for t in range(2):
                    # lhsT (KT slice) is reused by both q-halves: one
                    # LDWEIGHTS serves two matmuls.
                    for qh in range(2):
                        hs = slice(qh * MMN, (qh + 1) * MMN)
                        nc.tensor.matmul(stp[:, hs], KT[t][:, ksl],
                                         QT[t][:, q0b + qh * MMN:
                                                q0b + (qh + 1) * MMN],
                                         start=(t == 0), stop=(t == 1))
                if kt_i >= 4:
                    # PV lags four k-tiles: extra slack on the exp dependency.
                    kp = kt_i - 4
                    for j in range(n_qt):
                        # PSUM accumulation groups are bank-granular: two
                        # adjacent j-slices share a 2KB bank, so the group
                        # opens on the even slice and closes on the odd one.
                        nc.tensor.matmul(out_ps[:, j, :],
                                         elist[kp][:, j * P:(j + 1) * P],
                                         Vb[:, kp, :],
                                         start=(kp == 0 and j % 2 == 0),
                                         stop=False)
                ebf = work.tile([P, QB], BF16, tag="E", bufs=8)
                nc.scalar.activation(ebf, stp, AF.Exp, scale=scale)
                idx = kt_i % 2
                eng = nc.vector
                if kt_i < 2:
                    eng.tensor_copy(dacc[idx], ebf)
                else:
                    eng.tensor_add(dacc[idx], dacc[idx], ebf)
                elist.append(ebf)
            for kp in (n_kt - 4, n_kt - 3, n_kt - 2, n_kt - 1):
                for j in range(n_qt):
                    nc.tensor.matmul(out_ps[:, j, :],
                                     elist[kp][:, j * P:(j + 1) * P],
                                     Vb[:, kp, :],
                                     start=False, stop=(kp == n_kt - 1 and j % 2 == 1))

            dall = work.tile([P, QB], F32, tag="dall")
            nc.vector.tensor_add(dall, dacc[0], dacc[1])

            # softmax denominators via tiny N=1 matmuls: dn[:, j] =
            # dall_chunk_j.T @ ones -> [q, 1] columns, accumulated in a single
            # PSUM bank (group opens at j=0, closes at j=n_qt-1).
            dn_ps = ps_mm.tile([P, n_qt], F32, tag="mm", name=f"dn_{qb_i}")
            for j in range(n_qt):
                nc.tensor.matmul(dn_ps[:, j:j + 1],
                                 dall[:, j * P:(j + 1) * P], ones_f,
                                 start=(j == 0), stop=(j == n_qt - 1))
            recip = work.tile([P, n_qt], F32, tag="recip")
            nc.vector.reciprocal(recip, dn_ps)

            # finalize: out = out_ps*recip + bv, fused in one DVE op per tile
            for j in range(n_qt):
                res = outp.tile([P, EMB], F32, tag="res")
                nc.vector.scalar_tensor_tensor(
                    res, out_ps[:, j, :], recip[:, j:j + 1], bv_bc,
                    op0=mybir.AluOpType.mult, op1=mybir.AluOpType.add)
                q0 = q0b + j * P
                nc.sync.dma_start(out[q0:q0 + P, :], res)


def _make_nc(s_len: int = S) -> bass.Bass:
    # Bacc (not raw Bass): its compile() splits multi-sem waits and moves
    # matmul waits onto ldweights — HW allows at most one wait per inst.
    nc = bacc.Bacc("TRN2", target_bir_lowering=False, debug=False)
    _build(nc, s_len)
    nc.compile()
    return nc


def _prep(inputs: dict) -> dict:
    arrs = {k: np.ascontiguousarray(np.asarray(v, dtype=np.float32))
            for k, v in inputs.items()}
    assert arrs["x"].shape == (B, S, EMB), arrs["x"].shape
    return arrs


def run(inputs: dict):
    """Run on 8 NeuronCores. Returns (out[B,S,E] f32, BassKernelResults)."""
    arrs = _prep(inputs)
    nc = _make_nc(S)
    shared = {k: arrs[k] for k in ("Wq", "bq", "Wk", "bk", "Wv", "bv")}
    in_maps = [dict(shared, x=arrs["x"][i]) for i in range(B)]
    res = bass_utils.run_bass_kernel_spmd(nc, in_maps, core_ids=list(range(B)))
    out = np.stack([r["out"] for r in res.results], axis=0).astype(np.float32)
    return out, res


def kernel(**inputs) -> np.ndarray:
    out, _ = run(inputs)
    return out


def bench(inputs: dict, iters: int = 5, chain: int = 1):
    """Compile once, then time repeated executions with device-resident
    inputs (mirrors bass2jax.run_bass_via_pjrt's multi-core path).

    `chain` > 1 executes the NEFF that many times inside one XLA program
    (each call's outputs feed the next call's donated output buffers, which
    serializes them) so per-iteration device time can be extracted as a
    slope, amortizing the axon dispatch overhead.

    Returns (out[B,S,E] f32, list of per-call wall times in seconds).
    """
    import time

    import jax
    from jax.sharding import Mesh, NamedSharding, PartitionSpec
    from jax.experimental.shard_map import shard_map

    from concourse import bass2jax
    from concourse import mybir as mb

    arrs = _prep(inputs)
    nc = _make_nc(S)
    bass2jax.install_neuronx_cc_hook()

    partition_name = (
        nc.partition_id_tensor.name if nc.partition_id_tensor else None
    )
    in_names, out_names, out_avals, zero_outs = [], [], [], []
    for alloc in nc.m.functions[0].allocations:
        if not isinstance(alloc, mb.MemoryLocationSet):
            continue
        name = alloc.memorylocations[0].name
        if alloc.kind == "ExternalInput":
            if name != partition_name:
                in_names.append(name)
        elif alloc.kind == "ExternalOutput":
            out_names.append(name)
            shape = tuple(alloc.tensor_shape)
            dtype = mb.dt.np(alloc.dtype)
            out_avals.append(jax.core.ShapedArray(shape, dtype))
            zero_outs.append(np.zeros(shape, dtype))
    n_params = len(in_names)
    n_outs = len(out_avals)
    all_names = in_names + out_names
    if partition_name is not None:
        all_names = all_names + [partition_name]

    def _call(ins, zeros):
        operands = list(ins) + list(zeros)
        if partition_name is not None:
            operands.append(bass2jax.partition_id_tensor())
        return bass2jax._bass_exec_p.bind(
            *operands,
            out_avals=tuple(out_avals),
            in_names=tuple(all_names),
            out_names=tuple(out_names),
            lowering_input_output_aliases=(),
            sim_require_finite=True,
            sim_require_nnan=True,
            nc=nc,
        )

    def _body(*args):
        ins = list(args[:n_params])
        zeros = list(args[n_params:])
        outs = _call(ins, zeros)
        for _ in range(chain - 1):
            outs = _call(ins, list(outs))
        return tuple(outs)

    devices = jax.devices()[:B]
    mesh = Mesh(np.asarray(devices), ("core",))
    in_specs = (PartitionSpec("core"),) * (n_params + n_outs)
    out_specs = (PartitionSpec("core"),) * n_outs
    donate = tuple(range(n_params, n_params + n_outs))
    sharded = jax.jit(
        shard_map(_body, mesh=mesh, in_specs=in_specs, out_specs=out_specs,
                  check_rep=False),
        donate_argnums=donate,
        keep_unused=True,
    )

    per_core = [
        [arrs["x"][c] if n == "x" else arrs[n] for n in in_names[:n_params]]
        for c in range(B)
    ]
    concat_in = [
        np.concatenate([per_core[c][i] for c in range(B)], axis=0)
        for i in range(n_params)
    ]
    concat_zeros = [
        np.zeros((B * z.shape[0], *z.shape[1:]), z.dtype) for z in zero_outs
    ]

    shard = NamedSharding(mesh, PartitionSpec("core"))
    dev_in = [jax.device_put(a, shard) for a in concat_in]
    jax.block_until_ready(dev_in)

    times = []
    out_np = None
    for i in range(iters + 1):
        dev_zeros = [jax.device_put(z, shard) for z in concat_zeros]
        jax.block_until_ready(dev_zeros)
        t0 = time.perf_counter()
        outs = sharded(*dev_in, *dev_zeros)
        jax.block_until_ready(outs)
        dt = time.perf_counter() - t0
        if i == 0:
            idx = out_names.index("out")
            out_np = np.asarray(outs[idx]).reshape(B, S, EMB).astype(np.float32)
        else:
            times.append(dt)
    return out_np, times



# revision 3
# speedup vs baseline: 1.0327x; 1.0327x over previous
"""Single-head MHA (QKV proj + softmax attention) on 8 Trainium2 cores.

Problem: x[8, 4096, 256] f32; per-batch attention with per-head emb 256.
Sharding: data-parallel - one batch element per NeuronCore (8 cores).

Per-core algorithm (S=4096, E=256, P=128 partitions), all matmuls bf16:
  - A = Wq^T @ Wk [256, 256] once (tiny), so scores = (x @ A) @ x^T and the
    K projection disappears; the bq bias folds in exactly as a per-partition
    column u = bq @ Wk on the Q' projection, and the bk bias term is
    constant per q-row so it cancels in softmax.
  - x arrives in 5 batched DMAs; per 128-row tile: cast to bf16 (gpsimd),
    PE-transpose into xT[d, s], V-tile = xT.T @ WvT, and per 512 columns
    Q'T[e', s] = A.T @ xT (+u bias fused in the PSUM->SBUF copy).
  - attention per q-block of 1024 columns, two 512-wide halves per k-tile:
      S^T[k, qh] = xT_slice.T @ Q'T   (2 matmuls, fp32 PSUM, 1-bank tiles)
      E[k, qh]   = exp(S^T / 16)      (ScalarE, scale fused, bf16 out)
      out[q, e] += E_chunk.T @ V      (4 matmuls N=256 per half, lagged 4
                                       k-tiles; E q-chunks stationary so the
                                       output lands in [q, e] - no transposes)
      dn[q]     += E_chunk.T @ ones   (4 tiny N=1 matmuls per half into a
                                       dedicated PSUM bank: the softmax
                                       denominator costs no DVE time and is
                                       complete the moment the last exp is)
    finalize: recip (DVE), then out = out_ps*recip + bv per 128-row tile
    (softmax rows sum to 1, so attn @ (V + bv) = attn @ V + bv), alternating
    DVE / gpsimd, written to a staging tile and DMA'd out in 512-row blocks.
    No PE instruction depends on the finalize, so the PE streams straight
    into the next q-block.

PSUM budget exactly 8 banks: 3x[128,512]f32 score slots + [128,8,256]f32
PV accumulator (4 banks) + [128,8]f32 denominator bank.

No running-max subtraction: scores/16 ~ N(0,1); max observed ~10.5, exp
stays well inside fp32/bf16 range.
"""

from contextlib import ExitStack

import numpy as np

import concourse.bass as bass
import concourse.tile as tile
from concourse import bacc
from concourse import mybir
from concourse import bass_utils
from concourse.masks import make_identity

P = 128          # partitions
EMB = 256        # head dim
S = 4096         # sequence length
B = 8            # batch == number of cores
QB = 1024        # q-block
HB = 512         # q-half (one PSUM bank of fp32)

F32 = mybir.dt.float32
BF16 = mybir.dt.bfloat16
AF = mybir.ActivationFunctionType

X_BATCHES = (2, 6, 8, 8, 8)   # 128-row x tiles per input DMA


def _build(nc: bass.Bass, s_len: int = S) -> None:
    """Emit the per-core program into `nc` (SPMD: same program all cores)."""
    x = nc.dram_tensor("x", (s_len, EMB), F32, kind="ExternalInput").ap()
    Wq = nc.dram_tensor("Wq", (EMB, EMB), F32, kind="ExternalInput").ap()
    bq = nc.dram_tensor("bq", (EMB,), F32, kind="ExternalInput").ap()
    Wk = nc.dram_tensor("Wk", (EMB, EMB), F32, kind="ExternalInput").ap()
    Wv = nc.dram_tensor("Wv", (EMB, EMB), F32, kind="ExternalInput").ap()
    bv = nc.dram_tensor("bv", (EMB,), F32, kind="ExternalInput").ap()
    out = nc.dram_tensor("out", (s_len, EMB), F32, kind="ExternalOutput").ap()

    n_st = s_len // P      # 128-row tiles of the sequence
    n_qb = s_len // QB     # q-blocks
    n_kt = s_len // P      # k-tiles
    n_qt = QB // P         # 128-row q-tiles per q-block
    scale = float(EMB) ** -0.5

    with tile.TileContext(nc) as tc, ExitStack() as ctx:
        consts = ctx.enter_context(tc.tile_pool(name="consts", bufs=1))
        persist = ctx.enter_context(tc.tile_pool(name="persist", bufs=1))
        stage = ctx.enter_context(tc.tile_pool(name="stage", bufs=2))
        work = ctx.enter_context(tc.tile_pool(name="work", bufs=2))
        ps = ctx.enter_context(tc.tile_pool(name="ps", bufs=3, space="PSUM"))

        # ---- input DMAs first: everything else hides under them ----
        wq_st = stage.tile([P, 2, EMB], F32, tag="wst", bufs=3, name="wq_st")
        nc.sync.dma_start(wq_st, Wq.rearrange("(t p) m -> p t m", p=P))
        wk_st = stage.tile([P, 2, EMB], F32, tag="wst", bufs=3, name="wk_st")
        nc.sync.dma_start(wk_st, Wk.rearrange("(t p) m -> p t m", p=P))
        wv_st = stage.tile([P, 2, EMB], F32, tag="wst", bufs=3, name="wv_st")
        nc.sync.dma_start(wv_st, Wv.rearrange("(t p) m -> p t m", p=P))
        xst = []
        t0 = 0
        for bi, nb in enumerate(X_BATCHES):
            xb = stage.tile([P, nb, EMB], F32, tag="xst", name=f"xst{bi}")
            src = bass.AP(
                tensor=x.tensor, offset=x.offset + t0 * P * EMB,
                ap=[[EMB, P], [P * EMB, nb], [1, EMB]])
            nc.sync.dma_start(xb, src)
            xst.append(xb)
            t0 += nb
        bq_sb = consts.tile([P, 2], F32)
        nc.gpsimd.dma_start(bq_sb, bq.rearrange("(t p) -> p t", p=P))
        bv_bc = consts.tile([P, EMB], F32)
        nc.gpsimd.dma_start(
            bv_bc,
            bass.AP(tensor=bv.tensor, offset=bv.offset, ap=[[0, P], list(bv.ap[0])]),
        )

        # ---- constants ----
        idf = consts.tile([P, P], F32)
        make_identity(nc, idf)
        idb = consts.tile([P, P], BF16)
        nc.vector.tensor_copy(idb, idf)
        ones_f = consts.tile([P, 1], F32)
        nc.vector.memset(ones_f, 1.0)
        ones_bf = consts.tile([P, 1], BF16)
        nc.vector.memset(ones_bf, 1.0)

        # ---- weights: A = Wq^T @ Wk, u = bq @ Wk, WvT ----
        wq_bf = persist.tile([P, 2, EMB], BF16)
        nc.gpsimd.tensor_copy(wq_bf, wq_st)
        wk_bf = persist.tile([P, 2, EMB], BF16)
        nc.gpsimd.tensor_copy(wk_bf, wk_st)
        wv_bf = persist.tile([P, 2, EMB], BF16)
        nc.gpsimd.tensor_copy(wv_bf, wv_st)
        bq_bf = consts.tile([P, 2], BF16)
        nc.vector.tensor_copy(bq_bf, bq_sb)

        A_sb = persist.tile([P, 2, EMB], BF16)
        for dc in range(2):
            aps = ps.tile([P, EMB], F32, tag="sc", name=f"aps{dc}")
            for ec in range(2):
                nc.tensor.matmul(aps, wq_bf[:, ec, dc * P:(dc + 1) * P],
                                 wk_bf[:, ec, :], start=(ec == 0), stop=(ec == 1))
            nc.scalar.copy(A_sb[:, dc, :], aps)

        u_ps = ps.tile([1, EMB], F32, tag="sc")
        for ec in range(2):
            nc.tensor.matmul(u_ps, bq_bf[:, ec:ec + 1], wk_bf[:, ec, :],
                             start=(ec == 0), stop=(ec == 1))
        u_sb = work.tile([1, EMB], F32, tag="u_sb")
        nc.scalar.copy(u_sb, u_ps)
        u_col = consts.tile([P, 2], F32)
        for jc in range(2):
            utp = ps.tile([P, 1], F32, tag="sc", name=f"utp{jc}")
            nc.tensor.transpose(utp, u_sb[0:1, jc * P:(jc + 1) * P],
                                ones_f[0:1, 0:1])
            nc.scalar.copy(u_col[:, jc:jc + 1], utp)

        WvT = persist.tile([P, 2, EMB], BF16)
        for dc in range(2):
            for et in range(2):
                tp = ps.tile([P, P], BF16, tag="sc", name=f"wvtp{dc}{et}")
                nc.tensor.transpose(tp, wv_bf[:, et, dc * P:(dc + 1) * P], idb)
                nc.scalar.copy(WvT[:, dc, et * P:(et + 1) * P], tp)

        # ---- x: cast, PE-transpose -> xT[d, s]; project V and Q' ----
        xT = persist.tile([P, 2, s_len], BF16, name="xT")
        QpT = persist.tile([P, 2, s_len], BF16, name="QpT")
        Vb = persist.tile([P, n_st, EMB], BF16, name="Vb")
        st_i = 0
        for bi, nb in enumerate(X_BATCHES):
            xbf = stage.tile([P, nb, EMB], BF16, tag="xbf", name=f"xbf{bi}")
            nc.gpsimd.tensor_copy(xbf, xst[bi])
            for t in range(nb):
                ssl = slice(st_i * P, (st_i + 1) * P)
                tp = ps.tile([P, 2, P], BF16, tag="sc", name=f"xtp{st_i}")
                for dc in range(2):
                    nc.tensor.transpose(tp[:, dc, :],
                                        xbf[:, t, dc * P:(dc + 1) * P], idb)
                nc.vector.tensor_copy(xT[:, :, ssl], tp)
                vps = ps.tile([P, EMB], F32, tag="sc", name=f"vps{st_i}")
                for dc in range(2):
                    nc.tensor.matmul(vps, xT[:, dc, ssl], WvT[:, dc, :],
                                     start=(dc == 0), stop=(dc == 1))
                if st_i % 2 == 0:
                    nc.vector.tensor_copy(Vb[:, st_i, :], vps)
                else:
                    nc.scalar.copy(Vb[:, st_i, :], vps)
                st_i += 1
                # Q' projection per 512-column group, bias u fused
                if st_i % 4 == 0:
                    g = st_i // 4 - 1
                    gsl = slice(g * 4 * P, st_i * P)
                    for jc in range(2):
                        qps = ps.tile([P, HB], F32, tag="sc", name=f"qps{g}{jc}")
                        for dc in range(2):
                            nc.tensor.matmul(qps, A_sb[:, dc, jc * P:(jc + 1) * P],
                                             xT[:, dc, gsl],
                                             start=(dc == 0), stop=(dc == 1))
                        nc.scalar.activation(QpT[:, jc, gsl], qps, AF.Identity,
                                             bias=u_col[:, jc:jc + 1], scale=1.0)

        # ---- attention ----
        # q-blocks of (start, n_half) in 512-wide halves; the narrower final
        # blocks shorten the end-of-kernel drain (PV lag + finalize chain).
        qblocks = [(0, 2), (1024, 2), (2048, 2), (3072, 1), (3584, 1)]
        LAG = 2
        for qb_i, (q0b, n_h) in enumerate(qblocks):
            nq = n_h * 4   # 128-row q-tiles in this block
            # out accumulator in [q, e] layout: one PSUM tensor, 2 banks/half.
            out_ps = ps.tile([P, 8, EMB], F32, tag="po", bufs=1,
                             name=f"out_ps_{qb_i}")
            dn_ps = ps.tile([P, 8], F32, tag="dn", bufs=1, name=f"dn_{qb_i}")
            elist = []

            def emit_pv(kp):
                for h in range(n_h):
                    for j in range(4):
                        jg = h * 4 + j
                        nc.tensor.matmul(out_ps[:, jg, :],
                                         elist[kp][h][:, j * P:(j + 1) * P],
                                         Vb[:, kp, :],
                                         start=(kp == 0 and jg % 2 == 0),
                                         stop=(kp == n_kt - 1 and jg % 2 == 1))

            for kt_i in range(n_kt):
                ksl = slice(kt_i * P, (kt_i + 1) * P)
                epair = []
                for h in range(n_h):
                    hsl = slice(q0b + h * HB, q0b + (h + 1) * HB)
                    sc = ps.tile([P, HB], F32, tag="sc", name=f"sc{qb_i}_{kt_i}{h}")
                    for dc in range(2):
                        nc.tensor.matmul(sc, xT[:, dc, ksl], QpT[:, dc, hsl],
                                         start=(dc == 0), stop=(dc == 1))
                    ebf = work.tile([P, HB], BF16, tag="E", bufs=12,
                                    name=f"e{qb_i}_{kt_i}{h}")
                    nc.scalar.activation(ebf, sc, AF.Exp, scale=scale)
                    for j in range(4):
                        jg = h * 4 + j
                        nc.tensor.matmul(dn_ps[:, jg:jg + 1],
                                         ebf[:, j * P:(j + 1) * P], ones_bf,
                                         start=(kt_i == 0 and jg == 0),
                                         stop=(kt_i == n_kt - 1 and jg == nq - 1))
                    epair.append(ebf)
                elist.append(epair)
                if kt_i >= LAG:
                    emit_pv(kt_i - LAG)
            for kp in range(n_kt - LAG, n_kt):
                emit_pv(kp)

            # finalize: no PE dependency - PE streams into the next q-block
            recip = work.tile([P, 8], F32, tag="recip", name=f"recip{qb_i}")
            nc.vector.reciprocal(recip[:, 0:nq], dn_ps[:, 0:nq])
            ost = work.tile([P, 8, EMB], F32, tag="ost", name=f"ost{qb_i}")
            for j in range(nq):
                nc.vector.scalar_tensor_tensor(
                    ost[:, j, :], out_ps[:, j, :], recip[:, j:j + 1], bv_bc,
                    op0=mybir.AluOpType.mult, op1=mybir.AluOpType.add)
            for half in range(n_h):
                q0 = q0b + half * HB
                dst = bass.AP(
                    tensor=out.tensor, offset=out.offset + q0 * EMB,
                    ap=[[EMB, P], [P * EMB, 4], [1, EMB]])
                nc.sync.dma_start(dst, ost[:, half * 4:(half + 1) * 4, :])


def _make_nc(s_len: int = S) -> bass.Bass:
    # Bacc (not raw Bass): its compile() splits multi-sem waits and moves
    # matmul waits onto ldweights - HW allows at most one wait per inst.
    nc = bacc.Bacc("TRN2", target_bir_lowering=False, debug=False)
    _build(nc, s_len)
    nc.compile()
    return nc


def _prep(inputs: dict) -> dict:
    arrs = {k: np.ascontiguousarray(np.asarray(v, dtype=np.float32))
            for k, v in inputs.items()}
    assert arrs["x"].shape == (B, S, EMB), arrs["x"].shape
    return arrs


def run(inputs: dict):
    """Run on 8 NeuronCores. Returns (out[B,S,E] f32, BassKernelResults)."""
    arrs = _prep(inputs)
    nc = _make_nc(S)
    shared = {k: arrs[k] for k in ("Wq", "bq", "Wk", "Wv", "bv")}
    in_maps = [dict(shared, x=arrs["x"][i]) for i in range(B)]
    res = bass_utils.run_bass_kernel_spmd(nc, in_maps, core_ids=list(range(B)))
    out = np.stack([r["out"] for r in res.results], axis=0).astype(np.float32)
    return out, res


def kernel(**inputs) -> np.ndarray:
    out, _ = run(inputs)
    return out


def bench(inputs: dict, iters: int = 5, chain: int = 1):
    """Compile once, then time repeated executions with device-resident
    inputs (mirrors bass2jax.run_bass_via_pjrt's multi-core path).

    `chain` > 1 executes the NEFF that many times inside one XLA program
    (each call's outputs feed the next call's donated output buffers, which
    serializes them) so per-iteration device time can be extracted as a
    slope, amortizing the axon dispatch overhead.

    Returns (out[B,S,E] f32, list of per-call wall times in seconds).
    """
    import time

    import jax
    from jax.sharding import Mesh, NamedSharding, PartitionSpec
    from jax.experimental.shard_map import shard_map

    from concourse import bass2jax
    from concourse import mybir as mb

    arrs = _prep(inputs)
    nc = _make_nc(S)
    bass2jax.install_neuronx_cc_hook()

    partition_name = (
        nc.partition_id_tensor.name if nc.partition_id_tensor else None
    )
    in_names, out_names, out_avals, zero_outs = [], [], [], []
    for alloc in nc.m.functions[0].allocations:
        if not isinstance(alloc, mb.MemoryLocationSet):
            continue
        name = alloc.memorylocations[0].name
        if alloc.kind == "ExternalInput":
            if name != partition_name:
                in_names.append(name)
        elif alloc.kind == "ExternalOutput":
            out_names.append(name)
            shape = tuple(alloc.tensor_shape)
            dtype = mb.dt.np(alloc.dtype)
            out_avals.append(jax.core.ShapedArray(shape, dtype))
            zero_outs.append(np.zeros(shape, dtype))
    n_params = len(in_names)
    n_outs = len(out_avals)
    all_names = in_names + out_names
    if partition_name is not None:
        all_names = all_names + [partition_name]

    def _call(ins, zeros):
        operands = list(ins) + list(zeros)
        if partition_name is not None:
            operands.append(bass2jax.partition_id_tensor())
        return bass2jax._bass_exec_p.bind(
            *operands,
            out_avals=tuple(out_avals),
            in_names=tuple(all_names),
            out_names=tuple(out_names),
            lowering_input_output_aliases=(),
            sim_require_finite=True,
            sim_require_nnan=True,
            nc=nc,
        )

    def _body(*args):
        ins = list(args[:n_params])
        zeros = list(args[n_params:])
        outs = _call(ins, zeros)
        for _ in range(chain - 1):
            outs = _call(ins, list(outs))
        return tuple(outs)

    devices = jax.devices()[:B]
    mesh = Mesh(np.asarray(devices), ("core",))
    in_specs = (PartitionSpec("core"),) * (n_params + n_outs)
    out_specs = (PartitionSpec("core"),) * n_outs
    donate = tuple(range(n_params, n_params + n_outs))
    sharded = jax.jit(
        shard_map(_body, mesh=mesh, in_specs=in_specs, out_specs=out_specs,
                  check_rep=False),
        donate_argnums=donate,
        keep_unused=True,
    )

    per_core = [
        [arrs["x"][c] if n == "x" else arrs[n] for n in in_names[:n_params]]
        for c in range(B)
    ]
    concat_in = [
        np.concatenate([per_core[c][i] for c in range(B)], axis=0)
        for i in range(n_params)
    ]
    concat_zeros = [
        np.zeros((B * z.shape[0], *z.shape[1:]), z.dtype) for z in zero_outs
    ]

    shard = NamedSharding(mesh, PartitionSpec("core"))
    dev_in = [jax.device_put(a, shard) for a in concat_in]
    jax.block_until_ready(dev_in)

    times = []
    out_np = None
    for i in range(iters + 1):
        dev_zeros = [jax.device_put(z, shard) for z in concat_zeros]
        jax.block_until_ready(dev_zeros)
        t0 = time.perf_counter()
        outs = sharded(*dev_in, *dev_zeros)
        jax.block_until_ready(outs)
        dt = time.perf_counter() - t0
        if i == 0:
            idx = out_names.index("out")
            out_np = np.asarray(outs[idx]).reshape(B, S, EMB).astype(np.float32)
        else:
            times.append(dt)
    return out_np, times


# revision 8
# speedup vs baseline: 1.0454x; 1.0123x over previous
"""Single-head MHA (QKV proj + softmax attention) on 8 Trainium2 cores.

Problem: x[8, 4096, 256] f32; per-batch attention with per-head emb 256.
Sharding: data-parallel - one batch element per NeuronCore (8 cores).

Per-core algorithm (S=4096, E=256, P=128 partitions), all matmuls bf16:
  - A = Wq^T @ Wk [256, 256] once (tiny), so scores = (x @ A) @ x^T and the
    K projection disappears; the bq bias folds in exactly as a per-partition
    column u = bq @ Wk on the Q' projection, and the bk bias term is
    constant per q-row so it cancels in softmax.
  - x arrives in 5 batched DMAs; per 128-row tile: cast to bf16 (gpsimd),
    PE-transpose into xT[d, s], V-tile = xT.T @ WvT, and per 512 columns
    Q'T[e', s] = A.T @ xT (+u bias fused in the PSUM->SBUF copy).
  - attention per q-block of 1024 columns, two 512-wide halves per k-tile:
      S^T[k, qh] = xT_slice.T @ Q'T   (2 matmuls, fp32 PSUM, 1-bank tiles)
      E[k, qh]   = exp(S^T / 16)      (ScalarE, scale fused, bf16 out)
      out[q, e] += E_chunk.T @ V      (4 matmuls N=256 per half, lagged 4
                                       k-tiles; E q-chunks stationary so the
                                       output lands in [q, e] - no transposes)
      dn[q]     += E_chunk.T @ ones   (4 tiny N=1 matmuls per half into a
                                       dedicated PSUM bank: the softmax
                                       denominator costs no DVE time and is
                                       complete the moment the last exp is)
    finalize: recip (DVE), then out = out_ps*recip + bv per 128-row tile
    (softmax rows sum to 1, so attn @ (V + bv) = attn @ V + bv), alternating
    DVE / gpsimd, written to a staging tile and DMA'd out in 512-row blocks.
    No PE instruction depends on the finalize, so the PE streams straight
    into the next q-block.

PSUM budget exactly 8 banks: 3x[128,512]f32 score slots + [128,8,256]f32
PV accumulator (4 banks) + [128,8]f32 denominator bank.

No running-max subtraction: scores/16 ~ N(0,1); max observed ~10.5, exp
stays well inside fp32/bf16 range.
"""

from contextlib import ExitStack

import numpy as np

import concourse.bass as bass
import concourse.tile as tile
from concourse import bacc
from concourse import mybir
from concourse import bass_utils
from concourse.masks import make_identity

P = 128          # partitions
EMB = 256        # head dim
S = 4096         # sequence length
B = 8            # batch == number of cores
QB = 1024        # q-block
HB = 512         # q-half (one PSUM bank of fp32)

F32 = mybir.dt.float32
BF16 = mybir.dt.bfloat16
AF = mybir.ActivationFunctionType

X_BATCHES = (2, 6, 8, 8, 8)   # 128-row x tiles per input DMA


def _build(nc: bass.Bass, s_len: int = S) -> None:
    """Emit the per-core program into `nc` (SPMD: same program all cores)."""
    x = nc.dram_tensor("x", (s_len, EMB), F32, kind="ExternalInput").ap()
    Wq = nc.dram_tensor("Wq", (EMB, EMB), F32, kind="ExternalInput").ap()
    bq = nc.dram_tensor("bq", (EMB,), F32, kind="ExternalInput").ap()
    Wk = nc.dram_tensor("Wk", (EMB, EMB), F32, kind="ExternalInput").ap()
    Wv = nc.dram_tensor("Wv", (EMB, EMB), F32, kind="ExternalInput").ap()
    bv = nc.dram_tensor("bv", (EMB,), F32, kind="ExternalInput").ap()
    out = nc.dram_tensor("out", (s_len, EMB), F32, kind="ExternalOutput").ap()

    n_st = s_len // P      # 128-row tiles of the sequence
    n_qb = s_len // QB     # q-blocks
    n_kt = s_len // P      # k-tiles
    n_qt = QB // P         # 128-row q-tiles per q-block
    scale = float(EMB) ** -0.5

    with tile.TileContext(nc) as tc, ExitStack() as ctx:
        consts = ctx.enter_context(tc.tile_pool(name="consts", bufs=1))
        persist = ctx.enter_context(tc.tile_pool(name="persist", bufs=1))
        stage = ctx.enter_context(tc.tile_pool(name="stage", bufs=2))
        work = ctx.enter_context(tc.tile_pool(name="work", bufs=2))
        ps = ctx.enter_context(tc.tile_pool(name="ps", bufs=2, space="PSUM"))

        # ---- input DMAs first: everything else hides under them ----
        # order matters: HWDGE desc-gen and the DMA engines serialize, and
        # the critical chain to the first attention matmul is Wq+Wk -> A ->
        # Q' projection of the first x tiles.
        wq_st = stage.tile([P, 2, EMB], F32, tag="wst", bufs=3, name="wq_st")
        nc.sync.dma_start(wq_st, Wq.rearrange("(t p) m -> p t m", p=P))
        wk_st = stage.tile([P, 2, EMB], F32, tag="wst", bufs=3, name="wk_st")
        nc.sync.dma_start(wk_st, Wk.rearrange("(t p) m -> p t m", p=P))
        xst = []
        t0 = 0

        def dma_x_batch(bi):
            nonlocal t0
            nb = X_BATCHES[bi]
            xb = stage.tile([P, nb, EMB], F32, tag="xst", name=f"xst{bi}")
            src = bass.AP(
                tensor=x.tensor, offset=x.offset + t0 * P * EMB,
                ap=[[EMB, P], [P * EMB, nb], [1, EMB]])
            nc.sync.dma_start(xb, src)
            xst.append(xb)
            t0 += nb

        dma_x_batch(0)
        dma_x_batch(1)
        wv_st = stage.tile([P, 2, EMB], F32, tag="wst", bufs=3, name="wv_st")
        nc.sync.dma_start(wv_st, Wv.rearrange("(t p) m -> p t m", p=P))
        for bi in range(2, len(X_BATCHES)):
            dma_x_batch(bi)
        bq_sb = consts.tile([P, 2], F32)
        nc.gpsimd.dma_start(bq_sb, bq.rearrange("(t p) -> p t", p=P))
        bv_bc = consts.tile([P, EMB], F32)
        nc.gpsimd.dma_start(
            bv_bc,
            bass.AP(tensor=bv.tensor, offset=bv.offset, ap=[[0, P], list(bv.ap[0])]),
        )

        # ---- constants ----
        idf = consts.tile([P, P], F32)
        make_identity(nc, idf)
        idb = consts.tile([P, P], BF16)
        nc.vector.tensor_copy(idb, idf)
        ones_f = consts.tile([P, 1], F32)
        nc.vector.memset(ones_f, 1.0)

        # ---- weights: A = Wq^T @ Wk, u = bq @ Wk, WvT ----
        # Wq on DVE, Wk on Act: the casts run in parallel so A starts earliest
        wq_bf = persist.tile([P, 2, EMB], BF16)
        nc.vector.tensor_copy(wq_bf, wq_st)
        wk_bf = persist.tile([P, 2, EMB], BF16)
        nc.scalar.copy(wk_bf, wk_st)
        wv_bf = persist.tile([P, 2, EMB], BF16)
        nc.gpsimd.tensor_copy(wv_bf, wv_st)
        bq_bf = consts.tile([P, 2], BF16)
        nc.vector.tensor_copy(bq_bf, bq_sb)

        A_sb = persist.tile([P, 2, EMB], BF16)
        for dc in range(2):
            aps = ps.tile([P, EMB], F32, tag="sc", name=f"aps{dc}")
            for ec in range(2):
                nc.tensor.matmul(aps, wq_bf[:, ec, dc * P:(dc + 1) * P],
                                 wk_bf[:, ec, :], start=(ec == 0), stop=(ec == 1))
            nc.scalar.copy(A_sb[:, dc, :], aps)

        u_ps = ps.tile([1, EMB], F32, tag="sc")
        for ec in range(2):
            nc.tensor.matmul(u_ps, bq_bf[:, ec:ec + 1], wk_bf[:, ec, :],
                             start=(ec == 0), stop=(ec == 1))
        u_sb = work.tile([1, EMB], F32, tag="u_sb")
        nc.scalar.copy(u_sb, u_ps)
        u_col = consts.tile([P, 2], F32)
        for jc in range(2):
            utp = ps.tile([P, 1], F32, tag="sc", name=f"utp{jc}")
            nc.tensor.transpose(utp, u_sb[0:1, jc * P:(jc + 1) * P],
                                ones_f[0:1, 0:1])
            nc.scalar.copy(u_col[:, jc:jc + 1], utp)

        WvT = persist.tile([P, 2, EMB], BF16)
        for dc in range(2):
            for et in range(2):
                tp = ps.tile([P, P], BF16, tag="sc", name=f"wvtp{dc}{et}")
                nc.tensor.transpose(tp, wv_bf[:, et, dc * P:(dc + 1) * P], idb)
                nc.scalar.copy(WvT[:, dc, et * P:(et + 1) * P], tp)

        # ---- x: cast, PE-transpose -> xT[d, s]; project V and Q' ----
        # V(t) and Q'(group) are emitted one tile behind the transposes so
        # the PE never waits on the cross-engine xT SBUF copies.
        xT = persist.tile([P, 2, s_len], BF16, name="xT")
        QpT = persist.tile([P, 2, s_len], BF16, name="QpT")
        Vb = persist.tile([P, n_st, EMB], BF16, name="Vb")

        def emit_v(t):
            ssl = slice(t * P, (t + 1) * P)
            vps = ps.tile([P, EMB], F32, tag="sc", name=f"vps{t}")
            for dc in range(2):
                nc.tensor.matmul(vps, xT[:, dc, ssl], WvT[:, dc, :],
                                 start=(dc == 0), stop=(dc == 1))
            if t % 2 == 0:
                nc.vector.tensor_copy(Vb[:, t, :], vps)
            else:
                nc.scalar.copy(Vb[:, t, :], vps)

        def emit_qp(g):
            gsl = slice(g * 4 * P, (g + 1) * 4 * P)
            for jc in range(2):
                qps = ps.tile([P, HB], F32, tag="sc", name=f"qps{g}{jc}")
                for dc in range(2):
                    nc.tensor.matmul(qps, A_sb[:, dc, jc * P:(jc + 1) * P],
                                     xT[:, dc, gsl],
                                     start=(dc == 0), stop=(dc == 1))
                nc.scalar.activation(QpT[:, jc, gsl], qps, AF.Identity,
                                     bias=u_col[:, jc:jc + 1], scale=1.0)

        st_i = 0
        for bi, nb in enumerate(X_BATCHES):
            xbf = stage.tile([P, nb, EMB], BF16, tag="xbf", name=f"xbf{bi}")
            nc.gpsimd.tensor_copy(xbf, xst[bi])
            for t in range(nb):
                ssl = slice(st_i * P, (st_i + 1) * P)
                tp = ps.tile([P, 2, P], BF16, tag="sc", name=f"xtp{st_i}")
                for dc in range(2):
                    nc.tensor.transpose(tp[:, dc, :],
                                        xbf[:, t, dc * P:(dc + 1) * P], idb)
                nc.vector.tensor_copy(xT[:, :, ssl], tp)
                if st_i >= 1:
                    emit_v(st_i - 1)
                if st_i >= 4 and st_i % 4 == 0:
                    emit_qp(st_i // 4 - 1)
                st_i += 1
        emit_v(n_st - 1)
        emit_qp(n_st // 4 - 1)

        # ---- attention ----
        # q-blocks of (start, n_half) in 512-wide halves; the narrower final
        # blocks shorten the end-of-kernel drain (PV lag + finalize chain).
        # Per k-tile: ONE [128, n_h*512] PSUM score tile (bank per half), ONE
        # exp, ONE DVE denominator accumulate - minimizes the per-instruction
        # semaphore-wait overhead on the PE stream.
        qblocks = [(0, 2), (1024, 2), (2048, 2), (3072, 1), (3584, 1)]
        LAG = 2
        for qb_i, (q0b, n_h) in enumerate(qblocks):
            nq = n_h * 4   # 128-row q-tiles in this block
            out_ps = ps.tile([P, 8, EMB], F32, tag="po", bufs=1,
                             name=f"out_ps_{qb_i}")
            dacc = work.tile([P, n_h, HB], F32, tag="dacc", bufs=2,
                             name=f"dacc{qb_i}")
            elist = []

            def emit_pv(kp):
                for h in range(n_h):
                    for j in range(4):
                        jg = h * 4 + j
                        nc.tensor.matmul(out_ps[:, jg, :],
                                         elist[kp][:, h, j * P:(j + 1) * P],
                                         Vb[:, kp, :],
                                         start=(kp == 0 and jg % 2 == 0),
                                         stop=(kp == n_kt - 1 and jg % 2 == 1))

            for kt_i in range(n_kt):
                ksl = slice(kt_i * P, (kt_i + 1) * P)
                sc = ps.tile([P, n_h, HB], F32, tag="sc",
                             name=f"sc{qb_i}_{kt_i}")
                for dc in range(2):   # lhsT reused across halves: 1 LDWEIGHTS
                    for h in range(n_h):
                        hsl = slice(q0b + h * HB, q0b + (h + 1) * HB)
                        nc.tensor.matmul(sc[:, h, :], xT[:, dc, ksl],
                                         QpT[:, dc, hsl],
                                         start=(dc == 0), stop=(dc == 1))
                ebf = work.tile([P, n_h, HB], BF16, tag="E", bufs=6,
                                name=f"e{qb_i}_{kt_i}")
                nc.scalar.activation(ebf, sc, AF.Exp, scale=scale)
                if kt_i == 0:
                    nc.vector.tensor_copy(dacc, ebf)
                else:
                    nc.vector.tensor_add(dacc, dacc, ebf)
                elist.append(ebf)
                if kt_i >= LAG:
                    emit_pv(kt_i - LAG)
            for kp in range(n_kt - LAG, n_kt):
                emit_pv(kp)

            # denominators: tiny N=1 matmuls dacc_chunk.T @ ones -> [q, 1]
            # columns in one PSUM bank (an "sc" slot, free during the
            # boundary), then finalize with no PE dependency.
            dn_ps = ps.tile([P, nq], F32, tag="sc", name=f"dn_{qb_i}")
            for j in range(nq):
                nc.tensor.matmul(dn_ps[:, j:j + 1],
                                 dacc[:, j // 4, (j % 4) * P:(j % 4 + 1) * P],
                                 ones_f, start=(j == 0), stop=(j == nq - 1))
            recip = work.tile([P, 8], F32, tag="recip", name=f"recip{qb_i}")
            nc.vector.reciprocal(recip[:, 0:nq], dn_ps)
            ost = work.tile([P, 8, EMB], F32, tag="ost", name=f"ost{qb_i}")
            for j in range(nq):
                nc.vector.scalar_tensor_tensor(
                    ost[:, j, :], out_ps[:, j, :], recip[:, j:j + 1], bv_bc,
                    op0=mybir.AluOpType.mult, op1=mybir.AluOpType.add)
            for half in range(n_h):
                q0 = q0b + half * HB
                dst = bass.AP(
                    tensor=out.tensor, offset=out.offset + q0 * EMB,
                    ap=[[EMB, P], [P * EMB, 4], [1, EMB]])
                nc.sync.dma_start(dst, ost[:, half * 4:(half + 1) * 4, :])


def _make_nc(s_len: int = S) -> bass.Bass:
    # Bacc (not raw Bass): its compile() splits multi-sem waits and moves
    # matmul waits onto ldweights - HW allows at most one wait per inst.
    nc = bacc.Bacc("TRN2", target_bir_lowering=False, debug=False)
    _build(nc, s_len)
    nc.compile()
    return nc


def _prep(inputs: dict) -> dict:
    arrs = {k: np.ascontiguousarray(np.asarray(v, dtype=np.float32))
            for k, v in inputs.items()}
    assert arrs["x"].shape == (B, S, EMB), arrs["x"].shape
    return arrs


def run(inputs: dict):
    """Run on 8 NeuronCores. Returns (out[B,S,E] f32, BassKernelResults)."""
    arrs = _prep(inputs)
    nc = _make_nc(S)
    shared = {k: arrs[k] for k in ("Wq", "bq", "Wk", "Wv", "bv")}
    in_maps = [dict(shared, x=arrs["x"][i]) for i in range(B)]
    res = bass_utils.run_bass_kernel_spmd(nc, in_maps, core_ids=list(range(B)))
    out = np.stack([r["out"] for r in res.results], axis=0).astype(np.float32)
    return out, res


def kernel(**inputs) -> np.ndarray:
    out, _ = run(inputs)
    return out


def bench(inputs: dict, iters: int = 5, chain: int = 1):
    """Compile once, then time repeated executions with device-resident
    inputs (mirrors bass2jax.run_bass_via_pjrt's multi-core path).

    `chain` > 1 executes the NEFF that many times inside one XLA program
    (each call's outputs feed the next call's donated output buffers, which
    serializes them) so per-iteration device time can be extracted as a
    slope, amortizing the axon dispatch overhead.

    Returns (out[B,S,E] f32, list of per-call wall times in seconds).
    """
    import time

    import jax
    from jax.sharding import Mesh, NamedSharding, PartitionSpec
    from jax.experimental.shard_map import shard_map

    from concourse import bass2jax
    from concourse import mybir as mb

    arrs = _prep(inputs)
    nc = _make_nc(S)
    bass2jax.install_neuronx_cc_hook()

    partition_name = (
        nc.partition_id_tensor.name if nc.partition_id_tensor else None
    )
    in_names, out_names, out_avals, zero_outs = [], [], [], []
    for alloc in nc.m.functions[0].allocations:
        if not isinstance(alloc, mb.MemoryLocationSet):
            continue
        name = alloc.memorylocations[0].name
        if alloc.kind == "ExternalInput":
            if name != partition_name:
                in_names.append(name)
        elif alloc.kind == "ExternalOutput":
            out_names.append(name)
            shape = tuple(alloc.tensor_shape)
            dtype = mb.dt.np(alloc.dtype)
            out_avals.append(jax.core.ShapedArray(shape, dtype))
            zero_outs.append(np.zeros(shape, dtype))
    n_params = len(in_names)
    n_outs = len(out_avals)
    all_names = in_names + out_names
    if partition_name is not None:
        all_names = all_names + [partition_name]

    def _call(ins, zeros):
        operands = list(ins) + list(zeros)
        if partition_name is not None:
            operands.append(bass2jax.partition_id_tensor())
        return bass2jax._bass_exec_p.bind(
            *operands,
            out_avals=tuple(out_avals),
            in_names=tuple(all_names),
            out_names=tuple(out_names),
            lowering_input_output_aliases=(),
            sim_require_finite=True,
            sim_require_nnan=True,
            nc=nc,
        )

    def _body(*args):
        ins = list(args[:n_params])
        zeros = list(args[n_params:])
        outs = _call(ins, zeros)
        for _ in range(chain - 1):
            outs = _call(ins, list(outs))
        return tuple(outs)

    devices = jax.devices()[:B]
    mesh = Mesh(np.asarray(devices), ("core",))
    in_specs = (PartitionSpec("core"),) * (n_params + n_outs)
    out_specs = (PartitionSpec("core"),) * n_outs
    donate = tuple(range(n_params, n_params + n_outs))
    sharded = jax.jit(
        shard_map(_body, mesh=mesh, in_specs=in_specs, out_specs=out_specs,
                  check_rep=False),
        donate_argnums=donate,
        keep_unused=True,
    )

    per_core = [
        [arrs["x"][c] if n == "x" else arrs[n] for n in in_names[:n_params]]
        for c in range(B)
    ]
    concat_in = [
        np.concatenate([per_core[c][i] for c in range(B)], axis=0)
        for i in range(n_params)
    ]
    concat_zeros = [
        np.zeros((B * z.shape[0], *z.shape[1:]), z.dtype) for z in zero_outs
    ]

    shard = NamedSharding(mesh, PartitionSpec("core"))
    dev_in = [jax.device_put(a, shard) for a in concat_in]
    jax.block_until_ready(dev_in)

    times = []
    out_np = None
    for i in range(iters + 1):
        dev_zeros = [jax.device_put(z, shard) for z in concat_zeros]
        jax.block_until_ready(dev_zeros)
        t0 = time.perf_counter()
        outs = sharded(*dev_in, *dev_zeros)
        jax.block_until_ready(outs)
        dt = time.perf_counter() - t0
        if i == 0:
            idx = out_names.index("out")
            out_np = np.asarray(outs[idx]).reshape(B, S, EMB).astype(np.float32)
        else:
            times.append(dt)
    return out_np, times


# revision 11
# speedup vs baseline: 1.0544x; 1.0085x over previous
"""Single-head MHA (QKV proj + softmax attention) on 8 Trainium2 cores.

Problem: x[8, 4096, 256] f32; per-batch attention with per-head emb 256.
Sharding: data-parallel - one batch element per NeuronCore (8 cores).

Per-core algorithm (S=4096, E=256, P=128 partitions), all matmuls bf16:
  - A = Wq^T @ Wk [256, 256] once (tiny), so scores = (x @ A) @ x^T and the
    K projection disappears; the bq bias folds in exactly as a per-partition
    column u = bq @ Wk on the Q' projection, and the bk bias term is
    constant per q-row so it cancels in softmax.
  - x arrives in 5 batched DMAs; per 128-row tile: cast to bf16 (gpsimd),
    PE-transpose into xT[d, s], V-tile = xT.T @ WvT, and per 512 columns
    Q'T[e', s] = A.T @ xT (+u bias fused in the PSUM->SBUF copy).
  - attention per q-block of 1024 columns, two 512-wide halves per k-tile:
      S^T[k, qh] = xT_slice.T @ Q'T   (2 matmuls, fp32 PSUM, 1-bank tiles)
      E[k, qh]   = exp(S^T / 16)      (ScalarE, scale fused, bf16 out)
      out[q, e] += E_chunk.T @ V      (4 matmuls N=256 per half, lagged 4
                                       k-tiles; E q-chunks stationary so the
                                       output lands in [q, e] - no transposes)
      dn[q]     += E_chunk.T @ ones   (4 tiny N=1 matmuls per half into a
                                       dedicated PSUM bank: the softmax
                                       denominator costs no DVE time and is
                                       complete the moment the last exp is)
    finalize: recip (DVE), then out = out_ps*recip + bv per 128-row tile
    (softmax rows sum to 1, so attn @ (V + bv) = attn @ V + bv), alternating
    DVE / gpsimd, written to a staging tile and DMA'd out in 512-row blocks.
    No PE instruction depends on the finalize, so the PE streams straight
    into the next q-block.

PSUM budget exactly 8 banks: 3x[128,512]f32 score slots + [128,8,256]f32
PV accumulator (4 banks) + [128,8]f32 denominator bank.

No running-max subtraction: scores/16 ~ N(0,1); max observed ~10.5, exp
stays well inside fp32/bf16 range.
"""

from contextlib import ExitStack

import numpy as np

import concourse.bass as bass
import concourse.tile as tile
from concourse import bacc
from concourse import mybir
from concourse import bass_utils
from concourse.masks import make_identity

P = 128          # partitions
EMB = 256        # head dim
S = 4096         # sequence length
B = 8            # batch == number of cores
QB = 1024        # q-block
HB = 512         # q-half (one PSUM bank of fp32)

F32 = mybir.dt.float32
BF16 = mybir.dt.bfloat16
AF = mybir.ActivationFunctionType

X_BATCHES = (2, 6, 8, 8, 8)   # 128-row x tiles per input DMA


def _build(nc: bass.Bass, s_len: int = S) -> None:
    """Emit the per-core program into `nc` (SPMD: same program all cores)."""
    x = nc.dram_tensor("x", (s_len, EMB), F32, kind="ExternalInput").ap()
    Wq = nc.dram_tensor("Wq", (EMB, EMB), F32, kind="ExternalInput").ap()
    bq = nc.dram_tensor("bq", (EMB,), F32, kind="ExternalInput").ap()
    Wk = nc.dram_tensor("Wk", (EMB, EMB), F32, kind="ExternalInput").ap()
    Wv = nc.dram_tensor("Wv", (EMB, EMB), F32, kind="ExternalInput").ap()
    bv = nc.dram_tensor("bv", (EMB,), F32, kind="ExternalInput").ap()
    out = nc.dram_tensor("out", (s_len, EMB), F32, kind="ExternalOutput").ap()

    n_st = s_len // P      # 128-row tiles of the sequence
    n_qb = s_len // QB     # q-blocks
    n_kt = s_len // P      # k-tiles
    n_qt = QB // P         # 128-row q-tiles per q-block
    scale = float(EMB) ** -0.5

    with tile.TileContext(nc) as tc, ExitStack() as ctx:
        consts = ctx.enter_context(tc.tile_pool(name="consts", bufs=1))
        persist = ctx.enter_context(tc.tile_pool(name="persist", bufs=1))
        stage = ctx.enter_context(tc.tile_pool(name="stage", bufs=2))
        work = ctx.enter_context(tc.tile_pool(name="work", bufs=2))
        ps = ctx.enter_context(tc.tile_pool(name="ps", bufs=2, space="PSUM"))

        # ---- constants (no DMA deps: ready before the first transpose) ----
        idf = consts.tile([P, P], F32)
        make_identity(nc, idf)
        idb = consts.tile([P, P], BF16)
        nc.vector.tensor_copy(idb, idf)
        ones_f = consts.tile([P, 1], F32)
        nc.vector.memset(ones_f, 1.0)
        ones_bf = consts.tile([P, 1], BF16)
        nc.vector.memset(ones_bf, 1.0)

        # ---- input DMAs: everything else hides under them ----
        # order matters: HWDGE desc-gen and the DMA engines serialize; the
        # first PE work is x0 transposes, then A = f(Wq, Wk).
        xst = []
        t0 = 0

        def dma_x_batch(bi):
            nonlocal t0
            nb = X_BATCHES[bi]
            xb = stage.tile([P, nb, EMB], F32, tag="xst", name=f"xst{bi}")
            src = bass.AP(
                tensor=x.tensor, offset=x.offset + t0 * P * EMB,
                ap=[[EMB, P], [P * EMB, nb], [1, EMB]])
            nc.sync.dma_start(xb, src)
            xst.append(xb)
            t0 += nb

        dma_x_batch(0)
        wq_st = stage.tile([P, 2, EMB], F32, tag="wst", bufs=3, name="wq_st")
        nc.sync.dma_start(wq_st, Wq.rearrange("(t p) m -> p t m", p=P))
        wk_st = stage.tile([P, 2, EMB], F32, tag="wst", bufs=3, name="wk_st")
        nc.sync.dma_start(wk_st, Wk.rearrange("(t p) m -> p t m", p=P))
        dma_x_batch(1)
        wv_st = stage.tile([P, 2, EMB], F32, tag="wst", bufs=3, name="wv_st")
        nc.sync.dma_start(wv_st, Wv.rearrange("(t p) m -> p t m", p=P))
        bq_row = consts.tile([1, EMB], F32)
        nc.sync.dma_start(bq_row, bass.AP(tensor=bq.tensor, offset=bq.offset,
                                          ap=[[0, 1], list(bq.ap[0])]))
        bv_bc = consts.tile([P, EMB], F32)
        nc.sync.dma_start(
            bv_bc,
            bass.AP(tensor=bv.tensor, offset=bv.offset, ap=[[0, P], list(bv.ap[0])]),
        )
        for bi in range(2, len(X_BATCHES)):
            dma_x_batch(bi)

        # ---- weights: A = Wq^T @ Wk, u = bq @ Wk, WvT ----
        # Wq on DVE, Wk on Act: the casts run in parallel so A starts earliest
        wq_bf = persist.tile([P, 2, EMB], BF16)
        nc.vector.tensor_copy(wq_bf, wq_st)
        wk_bf = persist.tile([P, 2, EMB], BF16)
        nc.scalar.copy(wk_bf, wk_st)
        wv_bf = persist.tile([P, 2, EMB], BF16)
        nc.gpsimd.tensor_copy(wv_bf, wv_st)
        bq_bf = consts.tile([P, 2], BF16)
        for ec in range(2):
            btp = ps.tile([P, 1], F32, tag="sc", name=f"btp{ec}")
            nc.tensor.transpose(btp, bq_row[0:1, ec * P:(ec + 1) * P],
                                ones_f[0:1, 0:1])
            nc.scalar.copy(bq_bf[:, ec:ec + 1], btp)

        A_sb = persist.tile([P, 2, EMB], BF16)
        for dc in range(2):
            aps = ps.tile([P, EMB], F32, tag="sc", name=f"aps{dc}")
            for ec in range(2):
                nc.tensor.matmul(aps, wq_bf[:, ec, dc * P:(dc + 1) * P],
                                 wk_bf[:, ec, :], start=(ec == 0), stop=(ec == 1))
            nc.scalar.copy(A_sb[:, dc, :], aps)

        u_ps = ps.tile([1, EMB], F32, tag="sc")
        for ec in range(2):
            nc.tensor.matmul(u_ps, bq_bf[:, ec:ec + 1], wk_bf[:, ec, :],
                             start=(ec == 0), stop=(ec == 1))
        u_sb = work.tile([1, EMB], F32, tag="u_sb")
        nc.scalar.copy(u_sb, u_ps)
        u_col = consts.tile([P, 2], F32)
        for jc in range(2):
            utp = ps.tile([P, 1], F32, tag="sc", name=f"utp{jc}")
            nc.tensor.transpose(utp, u_sb[0:1, jc * P:(jc + 1) * P],
                                ones_f[0:1, 0:1])
            nc.scalar.copy(u_col[:, jc:jc + 1], utp)

        WvT = persist.tile([P, 2, EMB], BF16)
        for dc in range(2):
            for et in range(2):
                tp = ps.tile([P, P], BF16, tag="sc", name=f"wvtp{dc}{et}")
                nc.tensor.transpose(tp, wv_bf[:, et, dc * P:(dc + 1) * P], idb)
                nc.scalar.copy(WvT[:, dc, et * P:(et + 1) * P], tp)

        # ---- x: cast, PE-transpose -> xT[d, s]; project V and Q' ----
        # V(t) and Q'(group) are emitted one tile behind the transposes so
        # the PE never waits on the cross-engine xT SBUF copies.
        xT = persist.tile([P, 2, s_len], BF16, name="xT")
        QpT = persist.tile([P, 2, s_len], BF16, name="QpT")
        Vb = persist.tile([P, n_st, EMB], BF16, name="Vb")

        def emit_v(t):
            ssl = slice(t * P, (t + 1) * P)
            vps = ps.tile([P, EMB], F32, tag="sc", name=f"vps{t}")
            for dc in range(2):
                nc.tensor.matmul(vps, xT[:, dc, ssl], WvT[:, dc, :],
                                 start=(dc == 0), stop=(dc == 1))
            if t % 2 == 0:
                nc.vector.tensor_copy(Vb[:, t, :], vps)
            else:
                nc.scalar.copy(Vb[:, t, :], vps)

        def emit_qp(g):
            gsl = slice(g * 4 * P, (g + 1) * 4 * P)
            for jc in range(2):
                qps = ps.tile([P, HB], F32, tag="sc", name=f"qps{g}{jc}")
                for dc in range(2):
                    nc.tensor.matmul(qps, A_sb[:, dc, jc * P:(jc + 1) * P],
                                     xT[:, dc, gsl],
                                     start=(dc == 0), stop=(dc == 1))
                nc.scalar.activation(QpT[:, jc, gsl], qps, AF.Identity,
                                     bias=u_col[:, jc:jc + 1], scale=1.0)

        st_i = 0
        for bi, nb in enumerate(X_BATCHES):
            xbf = stage.tile([P, nb, EMB], BF16, tag="xbf", name=f"xbf{bi}")
            nc.gpsimd.tensor_copy(xbf, xst[bi])
            for t in range(nb):
                ssl = slice(st_i * P, (st_i + 1) * P)
                tp = ps.tile([P, 2, P], BF16, tag="sc", name=f"xtp{st_i}")
                for dc in range(2):
                    nc.tensor.transpose(tp[:, dc, :],
                                        xbf[:, t, dc * P:(dc + 1) * P], idb)
                nc.vector.tensor_copy(xT[:, :, ssl], tp)
                if st_i >= 1:
                    emit_v(st_i - 1)
                if st_i >= 4 and st_i % 4 == 0:
                    emit_qp(st_i // 4 - 1)
                st_i += 1
        emit_v(n_st - 1)
        emit_qp(n_st // 4 - 1)

        # ---- attention ----
        # q-blocks of (start, n_half) in 512-wide halves; the narrower final
        # blocks shorten the end-of-kernel drain (PV lag + finalize chain).
        # Per k-tile: ONE [128, n_h*512] PSUM score tile (bank per half), ONE
        # exp, ONE DVE denominator accumulate - minimizes the per-instruction
        # semaphore-wait overhead on the PE stream.
        qblocks = [(0, 2), (1024, 2), (2048, 2), (3072, 1), (3584, 1)]
        LAG = 2
        for qb_i, (q0b, n_h) in enumerate(qblocks):
            nq = n_h * 4   # 128-row q-tiles in this block
            out_ps = ps.tile([P, 8, EMB], F32, tag="po", bufs=1,
                             name=f"out_ps_{qb_i}")
            # two interleaved denominator accumulators (DVE + gpsimd) so
            # neither chain lags the PE and holds exp buffers alive
            dacc = [work.tile([P, n_h, HB], F32, tag=f"dacc{i}", bufs=2,
                              name=f"dacc{i}_{qb_i}") for i in range(2)]
            elist = []

            def emit_pv(kp):
                for h in range(n_h):
                    for j in range(4):
                        jg = h * 4 + j
                        nc.tensor.matmul(out_ps[:, jg, :],
                                         elist[kp][:, h, j * P:(j + 1) * P],
                                         Vb[:, kp, :],
                                         start=(kp == 0 and jg % 2 == 0),
                                         stop=(kp == n_kt - 1 and jg % 2 == 1))

            for kt_i in range(n_kt):
                ksl = slice(kt_i * P, (kt_i + 1) * P)
                sc = ps.tile([P, n_h, HB], F32, tag="sc",
                             name=f"sc{qb_i}_{kt_i}")
                for dc in range(2):   # lhsT reused across halves: 1 LDWEIGHTS
                    for h in range(n_h):
                        hsl = slice(q0b + h * HB, q0b + (h + 1) * HB)
                        nc.tensor.matmul(sc[:, h, :], xT[:, dc, ksl],
                                         QpT[:, dc, hsl],
                                         start=(dc == 0), stop=(dc == 1))
                ebf = work.tile([P, n_h, HB], BF16, tag="E", bufs=10,
                                name=f"e{qb_i}_{kt_i}")
                nc.scalar.activation(ebf, sc, AF.Exp, scale=scale)
                if kt_i < n_kt - 1:   # last tile's sum comes straight from ebf
                    ci = kt_i % 2
                    eng = nc.vector if ci == 0 else nc.gpsimd
                    da = dacc[ci]
                    if kt_i < 2:
                        eng.tensor_copy(da, ebf)
                    else:
                        eng.tensor_add(da, da, ebf)
                elist.append(ebf)
                if kt_i >= LAG:
                    emit_pv(kt_i - LAG)
            for kp in range(n_kt - LAG, n_kt):
                emit_pv(kp)

            # denominators: tiny N=1 matmuls chunk.T @ ones -> [q, 1] columns
            # in one PSUM bank (an "sc" slot, free during the boundary). The
            # last k-tile's term reads the exp tile directly so the chain
            # tails don't gate the finalize.
            dn_ps = ps.tile([P, nq], F32, tag="sc", name=f"dn_{qb_i}")
            srcs = [(dacc[0], ones_f), (dacc[1], ones_f), (elist[-1], ones_bf)]
            for si, (dsrc, drhs) in enumerate(srcs):
                for j in range(nq):
                    nc.tensor.matmul(
                        dn_ps[:, j:j + 1],
                        dsrc[:, j // 4, (j % 4) * P:(j % 4 + 1) * P], drhs,
                        start=(si == 0 and j == 0),
                        stop=(si == 2 and j == nq - 1))
            recip = work.tile([P, 8], F32, tag="recip", name=f"recip{qb_i}")
            nc.vector.reciprocal(recip[:, 0:nq], dn_ps)
            ost = work.tile([P, 8, EMB], F32, tag="ost", name=f"ost{qb_i}")
            for j in range(nq):
                nc.vector.scalar_tensor_tensor(
                    ost[:, j, :], out_ps[:, j, :], recip[:, j:j + 1], bv_bc,
                    op0=mybir.AluOpType.mult, op1=mybir.AluOpType.add)
            for half in range(n_h):
                q0 = q0b + half * HB
                dst = bass.AP(
                    tensor=out.tensor, offset=out.offset + q0 * EMB,
                    ap=[[EMB, P], [P * EMB, 4], [1, EMB]])
                nc.sync.dma_start(dst, ost[:, half * 4:(half + 1) * 4, :])


def _make_nc(s_len: int = S) -> bass.Bass:
    # Bacc (not raw Bass): its compile() splits multi-sem waits and moves
    # matmul waits onto ldweights - HW allows at most one wait per inst.
    nc = bacc.Bacc("TRN2", target_bir_lowering=False, debug=False)
    _build(nc, s_len)
    nc.compile()
    return nc


def _prep(inputs: dict) -> dict:
    arrs = {k: np.ascontiguousarray(np.asarray(v, dtype=np.float32))
            for k, v in inputs.items()}
    assert arrs["x"].shape == (B, S, EMB), arrs["x"].shape
    return arrs


def run(inputs: dict):
    """Run on 8 NeuronCores. Returns (out[B,S,E] f32, BassKernelResults)."""
    arrs = _prep(inputs)
    nc = _make_nc(S)
    shared = {k: arrs[k] for k in ("Wq", "bq", "Wk", "Wv", "bv")}
    in_maps = [dict(shared, x=arrs["x"][i]) for i in range(B)]
    res = bass_utils.run_bass_kernel_spmd(nc, in_maps, core_ids=list(range(B)))
    out = np.stack([r["out"] for r in res.results], axis=0).astype(np.float32)
    return out, res


def kernel(**inputs) -> np.ndarray:
    out, _ = run(inputs)
    return out


def bench(inputs: dict, iters: int = 5, chain: int = 1):
    """Compile once, then time repeated executions with device-resident
    inputs (mirrors bass2jax.run_bass_via_pjrt's multi-core path).

    `chain` > 1 executes the NEFF that many times inside one XLA program
    (each call's outputs feed the next call's donated output buffers, which
    serializes them) so per-iteration device time can be extracted as a
    slope, amortizing the axon dispatch overhead.

    Returns (out[B,S,E] f32, list of per-call wall times in seconds).
    """
    import time

    import jax
    from jax.sharding import Mesh, NamedSharding, PartitionSpec
    from jax.experimental.shard_map import shard_map

    from concourse import bass2jax
    from concourse import mybir as mb

    arrs = _prep(inputs)
    nc = _make_nc(S)
    bass2jax.install_neuronx_cc_hook()

    partition_name = (
        nc.partition_id_tensor.name if nc.partition_id_tensor else None
    )
    in_names, out_names, out_avals, zero_outs = [], [], [], []
    for alloc in nc.m.functions[0].allocations:
        if not isinstance(alloc, mb.MemoryLocationSet):
            continue
        name = alloc.memorylocations[0].name
        if alloc.kind == "ExternalInput":
            if name != partition_name:
                in_names.append(name)
        elif alloc.kind == "ExternalOutput":
            out_names.append(name)
            shape = tuple(alloc.tensor_shape)
            dtype = mb.dt.np(alloc.dtype)
            out_avals.append(jax.core.ShapedArray(shape, dtype))
            zero_outs.append(np.zeros(shape, dtype))
    n_params = len(in_names)
    n_outs = len(out_avals)
    all_names = in_names + out_names
    if partition_name is not None:
        all_names = all_names + [partition_name]

    def _call(ins, zeros):
        operands = list(ins) + list(zeros)
        if partition_name is not None:
            operands.append(bass2jax.partition_id_tensor())
        return bass2jax._bass_exec_p.bind(
            *operands,
            out_avals=tuple(out_avals),
            in_names=tuple(all_names),
            out_names=tuple(out_names),
            lowering_input_output_aliases=(),
            sim_require_finite=True,
            sim_require_nnan=True,
            nc=nc,
        )

    def _body(*args):
        ins = list(args[:n_params])
        zeros = list(args[n_params:])
        outs = _call(ins, zeros)
        for _ in range(chain - 1):
            outs = _call(ins, list(outs))
        return tuple(outs)

    devices = jax.devices()[:B]
    mesh = Mesh(np.asarray(devices), ("core",))
    in_specs = (PartitionSpec("core"),) * (n_params + n_outs)
    out_specs = (PartitionSpec("core"),) * n_outs
    donate = tuple(range(n_params, n_params + n_outs))
    sharded = jax.jit(
        shard_map(_body, mesh=mesh, in_specs=in_specs, out_specs=out_specs,
                  check_rep=False),
        donate_argnums=donate,
        keep_unused=True,
    )

    per_core = [
        [arrs["x"][c] if n == "x" else arrs[n] for n in in_names[:n_params]]
        for c in range(B)
    ]
    concat_in = [
        np.concatenate([per_core[c][i] for c in range(B)], axis=0)
        for i in range(n_params)
    ]
    concat_zeros = [
        np.zeros((B * z.shape[0], *z.shape[1:]), z.dtype) for z in zero_outs
    ]

    shard = NamedSharding(mesh, PartitionSpec("core"))
    dev_in = [jax.device_put(a, shard) for a in concat_in]
    jax.block_until_ready(dev_in)

    times = []
    out_np = None
    for i in range(iters + 1):
        dev_zeros = [jax.device_put(z, shard) for z in concat_zeros]
        jax.block_until_ready(dev_zeros)
        t0 = time.perf_counter()
        outs = sharded(*dev_in, *dev_zeros)
        jax.block_until_ready(outs)
        dt = time.perf_counter() - t0
        if i == 0:
            idx = out_names.index("out")
            out_np = np.asarray(outs[idx]).reshape(B, S, EMB).astype(np.float32)
        else:
            times.append(dt)
    return out_np, times


# revision 12
# speedup vs baseline: 1.0899x; 1.0337x over previous
"""Single-head MHA (QKV proj + softmax attention) on 8 Trainium2 cores.

Problem: x[8, 4096, 256] f32; per-batch attention with per-head emb 256.
Sharding: data-parallel - one batch element per NeuronCore (8 cores).

Per-core algorithm (S=4096, E=256, P=128 partitions), all matmuls bf16:
  - A = Wq^T @ Wk [256, 256] once (tiny), so scores = (x @ A) @ x^T and the
    K projection disappears; the bq bias folds in exactly as a per-partition
    column u = bq @ Wk on the Q' projection, and the bk bias term is
    constant per q-row so it cancels in softmax.
  - x arrives in 5 batched DMAs; per 128-row tile: cast to bf16 (gpsimd),
    PE-transpose into xT[d, s], V-tile = xT.T @ WvT, and per 512 columns
    Q'T[e', s] = A.T @ xT (+u bias fused in the PSUM->SBUF copy).
  - attention per q-block of 1024 columns, two 512-wide halves per k-tile:
      S^T[k, qh] = xT_slice.T @ Q'T   (2 matmuls, fp32 PSUM, 1-bank tiles)
      E[k, qh]   = exp(S^T / 16)      (ScalarE, scale fused, bf16 out)
      out[q, e] += E_chunk.T @ V      (4 matmuls N=256 per half, lagged 4
                                       k-tiles; E q-chunks stationary so the
                                       output lands in [q, e] - no transposes)
      dn[q]     += E_chunk.T @ ones   (4 tiny N=1 matmuls per half into a
                                       dedicated PSUM bank: the softmax
                                       denominator costs no DVE time and is
                                       complete the moment the last exp is)
    finalize: recip (DVE), then out = out_ps*recip + bv per 128-row tile
    (softmax rows sum to 1, so attn @ (V + bv) = attn @ V + bv), alternating
    DVE / gpsimd, written to a staging tile and DMA'd out in 512-row blocks.
    No PE instruction depends on the finalize, so the PE streams straight
    into the next q-block.

PSUM budget exactly 8 banks: 3x[128,512]f32 score slots + [128,8,256]f32
PV accumulator (4 banks) + [128,8]f32 denominator bank.

No running-max subtraction: scores/16 ~ N(0,1); max observed ~10.5, exp
stays well inside fp32/bf16 range.
"""

from contextlib import ExitStack

import numpy as np

import concourse.bass as bass
import concourse.tile as tile
from concourse import bacc
from concourse import mybir
from concourse import bass_utils
from concourse.masks import make_identity

P = 128          # partitions
EMB = 256        # head dim
S = 4096         # sequence length
B = 8            # batch == number of cores
QB = 1024        # q-block
HB = 512         # q-half (one PSUM bank of fp32)

F32 = mybir.dt.float32
BF16 = mybir.dt.bfloat16
AF = mybir.ActivationFunctionType

X_BATCHES = (4, 4, 8, 8, 8)   # 128-row x tiles per input DMA


def _build(nc: bass.Bass, s_len: int = S) -> None:
    """Emit the per-core program into `nc` (SPMD: same program all cores)."""
    x = nc.dram_tensor("x", (s_len, EMB), F32, kind="ExternalInput").ap()
    Wq = nc.dram_tensor("Wq", (EMB, EMB), F32, kind="ExternalInput").ap()
    bq = nc.dram_tensor("bq", (EMB,), F32, kind="ExternalInput").ap()
    Wk = nc.dram_tensor("Wk", (EMB, EMB), F32, kind="ExternalInput").ap()
    Wv = nc.dram_tensor("Wv", (EMB, EMB), F32, kind="ExternalInput").ap()
    bv = nc.dram_tensor("bv", (EMB,), F32, kind="ExternalInput").ap()
    out = nc.dram_tensor("out", (s_len, EMB), F32, kind="ExternalOutput").ap()

    n_st = s_len // P      # 128-row tiles of the sequence
    n_qb = s_len // QB     # q-blocks
    n_kt = s_len // P      # k-tiles
    n_qt = QB // P         # 128-row q-tiles per q-block
    scale = float(EMB) ** -0.5

    with tile.TileContext(nc) as tc, ExitStack() as ctx:
        consts = ctx.enter_context(tc.tile_pool(name="consts", bufs=1))
        persist = ctx.enter_context(tc.tile_pool(name="persist", bufs=1))
        stage = ctx.enter_context(tc.tile_pool(name="stage", bufs=2))
        work = ctx.enter_context(tc.tile_pool(name="work", bufs=2))
        ps = ctx.enter_context(tc.tile_pool(name="ps", bufs=2, space="PSUM"))

        # ---- constants (no DMA deps: ready before the first transpose) ----
        idf = consts.tile([P, P], F32)
        make_identity(nc, idf)
        idb = consts.tile([P, P], BF16)
        nc.vector.tensor_copy(idb, idf)
        ones_f = consts.tile([P, 1], F32)
        nc.vector.memset(ones_f, 1.0)
        ones_bf = consts.tile([P, 1], BF16)
        nc.vector.memset(ones_bf, 1.0)

        # ---- input DMAs: everything else hides under them ----
        # order matters: HWDGE desc-gen and the DMA engines serialize; the
        # first PE work is x0 transposes, then A = f(Wq, Wk).
        xst = []
        t0 = 0

        def dma_x_batch(bi):
            nonlocal t0
            nb = X_BATCHES[bi]
            xb = stage.tile([P, nb, EMB], F32, tag="xst", name=f"xst{bi}")
            src = bass.AP(
                tensor=x.tensor, offset=x.offset + t0 * P * EMB,
                ap=[[EMB, P], [P * EMB, nb], [1, EMB]])
            nc.sync.dma_start(xb, src)
            xst.append(xb)
            t0 += nb

        dma_x_batch(0)
        wq_st = stage.tile([P, 2, EMB], F32, tag="wst", bufs=3, name="wq_st")
        nc.sync.dma_start(wq_st, Wq.rearrange("(t p) m -> p t m", p=P))
        wk_st = stage.tile([P, 2, EMB], F32, tag="wst", bufs=3, name="wk_st")
        nc.sync.dma_start(wk_st, Wk.rearrange("(t p) m -> p t m", p=P))
        wv_st = stage.tile([P, 2, EMB], F32, tag="wst", bufs=3, name="wv_st")
        nc.sync.dma_start(wv_st, Wv.rearrange("(t p) m -> p t m", p=P))
        dma_x_batch(1)
        dma_x_batch(2)
        bq_row = consts.tile([1, EMB], F32)
        nc.sync.dma_start(bq_row, bass.AP(tensor=bq.tensor, offset=bq.offset,
                                          ap=[[0, 1], list(bq.ap[0])]))
        bv_bc = consts.tile([P, EMB], F32)
        nc.sync.dma_start(
            bv_bc,
            bass.AP(tensor=bv.tensor, offset=bv.offset, ap=[[0, P], list(bv.ap[0])]),
        )
        for bi in range(3, len(X_BATCHES)):
            dma_x_batch(bi)

        # ---- weights: A = Wq^T @ Wk, u = bq @ Wk, WvT ----
        # Wq on DVE, Wk on Act: the casts run in parallel so A starts earliest
        wq_bf = persist.tile([P, 2, EMB], BF16)
        nc.vector.tensor_copy(wq_bf, wq_st)
        wk_bf = persist.tile([P, 2, EMB], BF16)
        nc.scalar.copy(wk_bf, wk_st)
        wv_bf = persist.tile([P, 2, EMB], BF16)
        nc.scalar.copy(wv_bf, wv_st)
        bq_bf = consts.tile([P, 2], BF16)
        for ec in range(2):
            btp = ps.tile([P, 1], F32, tag="sc", name=f"btp{ec}")
            nc.tensor.transpose(btp, bq_row[0:1, ec * P:(ec + 1) * P],
                                ones_f[0:1, 0:1])
            nc.scalar.copy(bq_bf[:, ec:ec + 1], btp)

        A_sb = persist.tile([P, 2, EMB], BF16)
        for dc in range(2):
            aps = ps.tile([P, EMB], F32, tag="sc", name=f"aps{dc}")
            for ec in range(2):
                nc.tensor.matmul(aps, wq_bf[:, ec, dc * P:(dc + 1) * P],
                                 wk_bf[:, ec, :], start=(ec == 0), stop=(ec == 1))
            nc.scalar.copy(A_sb[:, dc, :], aps)

        u_ps = ps.tile([1, EMB], F32, tag="sc")
        for ec in range(2):
            nc.tensor.matmul(u_ps, bq_bf[:, ec:ec + 1], wk_bf[:, ec, :],
                             start=(ec == 0), stop=(ec == 1))
        u_sb = work.tile([1, EMB], F32, tag="u_sb")
        nc.scalar.copy(u_sb, u_ps)
        u_col = consts.tile([P, 2], F32)
        for jc in range(2):
            utp = ps.tile([P, 1], F32, tag="sc", name=f"utp{jc}")
            nc.tensor.transpose(utp, u_sb[0:1, jc * P:(jc + 1) * P],
                                ones_f[0:1, 0:1])
            nc.scalar.copy(u_col[:, jc:jc + 1], utp)

        WvT = persist.tile([P, 2, EMB], BF16)
        for dc in range(2):
            for et in range(2):
                tp = ps.tile([P, P], BF16, tag="sc", name=f"wvtp{dc}{et}")
                nc.tensor.transpose(tp, wv_bf[:, et, dc * P:(dc + 1) * P], idb)
                nc.scalar.copy(WvT[:, dc, et * P:(et + 1) * P], tp)

        # ---- x: cast, PE-transpose -> xT[d, s]; project V and Q' ----
        # 4-tile groups share one PSUM tile per stage (transposes, V, Q') so
        # the 2-slot PSUM rotation amortizes the cross-engine copy latency;
        # V(g) and Q'(g) trail the transposes of group g+1.
        xT = persist.tile([P, 2, s_len], BF16, name="xT")
        QpT = persist.tile([P, 2, s_len], BF16, name="QpT")
        Vb = persist.tile([P, n_st, EMB], BF16, name="Vb")
        GT = 4   # tiles per group

        def emit_vqp(g):
            gsl = slice(g * GT * P, (g + 1) * GT * P)
            vB = ps.tile([P, GT, EMB], F32, tag="sc", name=f"vB{g}")
            for tl in range(GT):
                tsl = slice((g * GT + tl) * P, (g * GT + tl + 1) * P)
                for dc in range(2):
                    nc.tensor.matmul(vB[:, tl, :], xT[:, dc, tsl], WvT[:, dc, :],
                                     start=(dc == 0), stop=(dc == 1))
            if g % 2 == 0:
                nc.vector.tensor_copy(Vb[:, g * GT:(g + 1) * GT, :], vB)
            else:
                nc.scalar.copy(Vb[:, g * GT:(g + 1) * GT, :], vB)
            qpB = ps.tile([P, 2, HB], F32, tag="sc", name=f"qpB{g}")
            for jc in range(2):
                for dc in range(2):
                    nc.tensor.matmul(qpB[:, jc, :], A_sb[:, dc, jc * P:(jc + 1) * P],
                                     xT[:, dc, gsl],
                                     start=(dc == 0), stop=(dc == 1))
            for jc in range(2):
                nc.scalar.activation(QpT[:, jc, gsl], qpB[:, jc, :], AF.Identity,
                                     bias=u_col[:, jc:jc + 1], scale=1.0)

        g_i = 0
        for bi, nb in enumerate(X_BATCHES):
            xbf = stage.tile([P, nb, EMB], BF16, tag="xbf", name=f"xbf{bi}")
            nc.gpsimd.tensor_copy(xbf, xst[bi])
            for t0g in range(0, nb, GT):
                gsl = slice(g_i * GT * P, (g_i + 1) * GT * P)
                tpB = ps.tile([P, 2, GT * P], BF16, tag="sc", name=f"tpB{g_i}")
                for tl in range(GT):
                    for dc in range(2):
                        nc.tensor.transpose(
                            tpB[:, dc, tl * P:(tl + 1) * P],
                            xbf[:, t0g + tl, dc * P:(dc + 1) * P], idb)
                nc.vector.tensor_copy(xT[:, :, gsl], tpB)
                if g_i >= 1:
                    emit_vqp(g_i - 1)
                g_i += 1
        emit_vqp(g_i - 1)

        # ---- attention ----
        # q-blocks of (start, n_half) in 512-wide halves; the narrower final
        # blocks shorten the end-of-kernel drain (PV lag + finalize chain).
        # Per k-tile: ONE [128, n_h*512] PSUM score tile (bank per half), ONE
        # exp, ONE DVE denominator accumulate - minimizes the per-instruction
        # semaphore-wait overhead on the PE stream.
        qblocks = [(0, 2), (1024, 2), (2048, 2), (3072, 1), (3584, 1)]
        LAG = 2
        for qb_i, (q0b, n_h) in enumerate(qblocks):
            nq = n_h * 4   # 128-row q-tiles in this block
            out_ps = ps.tile([P, 8, EMB], F32, tag="po", bufs=1,
                             name=f"out_ps_{qb_i}")
            # two interleaved denominator accumulators (DVE + gpsimd) so
            # neither chain lags the PE and holds exp buffers alive
            dacc = [work.tile([P, n_h, HB], F32, tag=f"dacc{i}", bufs=2,
                              name=f"dacc{i}_{qb_i}") for i in range(2)]
            elist = []

            def emit_pv(kp):
                for h in range(n_h):
                    for j in range(4):
                        jg = h * 4 + j
                        nc.tensor.matmul(out_ps[:, jg, :],
                                         elist[kp][:, h, j * P:(j + 1) * P],
                                         Vb[:, kp, :],
                                         start=(kp == 0 and jg % 2 == 0),
                                         stop=(kp == n_kt - 1 and jg % 2 == 1))

            for kt_i in range(n_kt):
                ksl = slice(kt_i * P, (kt_i + 1) * P)
                sc = ps.tile([P, n_h, HB], F32, tag="sc",
                             name=f"sc{qb_i}_{kt_i}")
                for dc in range(2):   # lhsT reused across halves: 1 LDWEIGHTS
                    for h in range(n_h):
                        hsl = slice(q0b + h * HB, q0b + (h + 1) * HB)
                        nc.tensor.matmul(sc[:, h, :], xT[:, dc, ksl],
                                         QpT[:, dc, hsl],
                                         start=(dc == 0), stop=(dc == 1))
                ebf = work.tile([P, n_h, HB], BF16, tag="E", bufs=10,
                                name=f"e{qb_i}_{kt_i}")
                nc.scalar.activation(ebf, sc, AF.Exp, scale=scale)
                if kt_i < n_kt - 1:   # last tile's sum comes straight from ebf
                    ci = kt_i % 2
                    eng = nc.vector if ci == 0 else nc.gpsimd
                    da = dacc[ci]
                    if kt_i < 2:
                        eng.tensor_copy(da, ebf)
                    else:
                        eng.tensor_add(da, da, ebf)
                elist.append(ebf)
                if kt_i >= LAG:
                    emit_pv(kt_i - LAG)
            for kp in range(n_kt - LAG, n_kt):
                emit_pv(kp)

            # denominators: tiny N=1 matmuls chunk.T @ ones -> [q, 1] columns
            # in one PSUM bank (an "sc" slot, free during the boundary). The
            # last k-tile's term reads the exp tile directly so the chain
            # tails don't gate the finalize.
            dn_ps = ps.tile([P, nq], F32, tag="sc", name=f"dn_{qb_i}")
            srcs = [(dacc[0], ones_f), (dacc[1], ones_f), (elist[-1], ones_bf)]
            for si, (dsrc, drhs) in enumerate(srcs):
                for j in range(nq):
                    nc.tensor.matmul(
                        dn_ps[:, j:j + 1],
                        dsrc[:, j // 4, (j % 4) * P:(j % 4 + 1) * P], drhs,
                        start=(si == 0 and j == 0),
                        stop=(si == 2 and j == nq - 1))
            recip = work.tile([P, 8], F32, tag="recip", name=f"recip{qb_i}")
            nc.vector.reciprocal(recip[:, 0:nq], dn_ps)
            ost = work.tile([P, 8, EMB], F32, tag="ost", name=f"ost{qb_i}")
            for j in range(nq):
                nc.vector.scalar_tensor_tensor(
                    ost[:, j, :], out_ps[:, j, :], recip[:, j:j + 1], bv_bc,
                    op0=mybir.AluOpType.mult, op1=mybir.AluOpType.add)
            for half in range(n_h):
                q0 = q0b + half * HB
                dst = bass.AP(
                    tensor=out.tensor, offset=out.offset + q0 * EMB,
                    ap=[[EMB, P], [P * EMB, 4], [1, EMB]])
                nc.sync.dma_start(dst, ost[:, half * 4:(half + 1) * 4, :])


def _make_nc(s_len: int = S) -> bass.Bass:
    # Bacc (not raw Bass): its compile() splits multi-sem waits and moves
    # matmul waits onto ldweights - HW allows at most one wait per inst.
    nc = bacc.Bacc("TRN2", target_bir_lowering=False, debug=False)
    _build(nc, s_len)
    nc.compile()
    return nc


def _prep(inputs: dict) -> dict:
    arrs = {k: np.ascontiguousarray(np.asarray(v, dtype=np.float32))
            for k, v in inputs.items()}
    assert arrs["x"].shape == (B, S, EMB), arrs["x"].shape
    return arrs


def run(inputs: dict):
    """Run on 8 NeuronCores. Returns (out[B,S,E] f32, BassKernelResults)."""
    arrs = _prep(inputs)
    nc = _make_nc(S)
    shared = {k: arrs[k] for k in ("Wq", "bq", "Wk", "Wv", "bv")}
    in_maps = [dict(shared, x=arrs["x"][i]) for i in range(B)]
    res = bass_utils.run_bass_kernel_spmd(nc, in_maps, core_ids=list(range(B)))
    out = np.stack([r["out"] for r in res.results], axis=0).astype(np.float32)
    return out, res


def kernel(**inputs) -> np.ndarray:
    out, _ = run(inputs)
    return out


def bench(inputs: dict, iters: int = 5, chain: int = 1):
    """Compile once, then time repeated executions with device-resident
    inputs (mirrors bass2jax.run_bass_via_pjrt's multi-core path).

    `chain` > 1 executes the NEFF that many times inside one XLA program
    (each call's outputs feed the next call's donated output buffers, which
    serializes them) so per-iteration device time can be extracted as a
    slope, amortizing the axon dispatch overhead.

    Returns (out[B,S,E] f32, list of per-call wall times in seconds).
    """
    import time

    import jax
    from jax.sharding import Mesh, NamedSharding, PartitionSpec
    from jax.experimental.shard_map import shard_map

    from concourse import bass2jax
    from concourse import mybir as mb

    arrs = _prep(inputs)
    nc = _make_nc(S)
    bass2jax.install_neuronx_cc_hook()

    partition_name = (
        nc.partition_id_tensor.name if nc.partition_id_tensor else None
    )
    in_names, out_names, out_avals, zero_outs = [], [], [], []
    for alloc in nc.m.functions[0].allocations:
        if not isinstance(alloc, mb.MemoryLocationSet):
            continue
        name = alloc.memorylocations[0].name
        if alloc.kind == "ExternalInput":
            if name != partition_name:
                in_names.append(name)
        elif alloc.kind == "ExternalOutput":
            out_names.append(name)
            shape = tuple(alloc.tensor_shape)
            dtype = mb.dt.np(alloc.dtype)
            out_avals.append(jax.core.ShapedArray(shape, dtype))
            zero_outs.append(np.zeros(shape, dtype))
    n_params = len(in_names)
    n_outs = len(out_avals)
    all_names = in_names + out_names
    if partition_name is not None:
        all_names = all_names + [partition_name]

    def _call(ins, zeros):
        operands = list(ins) + list(zeros)
        if partition_name is not None:
            operands.append(bass2jax.partition_id_tensor())
        return bass2jax._bass_exec_p.bind(
            *operands,
            out_avals=tuple(out_avals),
            in_names=tuple(all_names),
            out_names=tuple(out_names),
            lowering_input_output_aliases=(),
            sim_require_finite=True,
            sim_require_nnan=True,
            nc=nc,
        )

    def _body(*args):
        ins = list(args[:n_params])
        zeros = list(args[n_params:])
        outs = _call(ins, zeros)
        for _ in range(chain - 1):
            outs = _call(ins, list(outs))
        return tuple(outs)

    devices = jax.devices()[:B]
    mesh = Mesh(np.asarray(devices), ("core",))
    in_specs = (PartitionSpec("core"),) * (n_params + n_outs)
    out_specs = (PartitionSpec("core"),) * n_outs
    donate = tuple(range(n_params, n_params + n_outs))
    sharded = jax.jit(
        shard_map(_body, mesh=mesh, in_specs=in_specs, out_specs=out_specs,
                  check_rep=False),
        donate_argnums=donate,
        keep_unused=True,
    )

    per_core = [
        [arrs["x"][c] if n == "x" else arrs[n] for n in in_names[:n_params]]
        for c in range(B)
    ]
    concat_in = [
        np.concatenate([per_core[c][i] for c in range(B)], axis=0)
        for i in range(n_params)
    ]
    concat_zeros = [
        np.zeros((B * z.shape[0], *z.shape[1:]), z.dtype) for z in zero_outs
    ]

    shard = NamedSharding(mesh, PartitionSpec("core"))
    dev_in = [jax.device_put(a, shard) for a in concat_in]
    jax.block_until_ready(dev_in)

    times = []
    out_np = None
    for i in range(iters + 1):
        dev_zeros = [jax.device_put(z, shard) for z in concat_zeros]
        jax.block_until_ready(dev_zeros)
        t0 = time.perf_counter()
        outs = sharded(*dev_in, *dev_zeros)
        jax.block_until_ready(outs)
        dt = time.perf_counter() - t0
        if i == 0:
            idx = out_names.index("out")
            out_np = np.asarray(outs[idx]).reshape(B, S, EMB).astype(np.float32)
        else:
            times.append(dt)
    return out_np, times


# revision 13
# speedup vs baseline: 1.1035x; 1.0125x over previous
"""Single-head MHA (QKV proj + softmax attention) on 8 Trainium2 cores.

Problem: x[8, 4096, 256] f32; per-batch attention with per-head emb 256.
Sharding: data-parallel - one batch element per NeuronCore (8 cores).

Per-core algorithm (S=4096, E=256, P=128 partitions), all matmuls bf16:
  - A = Wq^T @ Wk [256, 256] once (tiny), so scores = (x @ A) @ x^T and the
    K projection disappears; the bq bias folds in exactly as a per-partition
    column u = bq @ Wk on the Q' projection, and the bk bias term is
    constant per q-row so it cancels in softmax.
  - x arrives in 5 batched DMAs; per 128-row tile: cast to bf16 (gpsimd),
    PE-transpose into xT[d, s], V-tile = xT.T @ WvT, and per 512 columns
    Q'T[e', s] = A.T @ xT (+u bias fused in the PSUM->SBUF copy).
  - attention per q-block of 1024 columns, two 512-wide halves per k-tile:
      S^T[k, qh] = xT_slice.T @ Q'T   (2 matmuls, fp32 PSUM, 1-bank tiles)
      E[k, qh]   = exp(S^T / 16)      (ScalarE, scale fused, bf16 out)
      out[q, e] += E_chunk.T @ V      (4 matmuls N=256 per half, lagged 4
                                       k-tiles; E q-chunks stationary so the
                                       output lands in [q, e] - no transposes)
      dn[q]     += E_chunk.T @ ones   (4 tiny N=1 matmuls per half into a
                                       dedicated PSUM bank: the softmax
                                       denominator costs no DVE time and is
                                       complete the moment the last exp is)
    finalize: recip (DVE), then out = out_ps*recip + bv per 128-row tile
    (softmax rows sum to 1, so attn @ (V + bv) = attn @ V + bv), alternating
    DVE / gpsimd, written to a staging tile and DMA'd out in 512-row blocks.
    No PE instruction depends on the finalize, so the PE streams straight
    into the next q-block.

PSUM budget exactly 8 banks: 3x[128,512]f32 score slots + [128,8,256]f32
PV accumulator (4 banks) + [128,8]f32 denominator bank.

No running-max subtraction: scores/16 ~ N(0,1); max observed ~10.5, exp
stays well inside fp32/bf16 range.
"""

from contextlib import ExitStack

import numpy as np

import concourse.bass as bass
import concourse.tile as tile
from concourse import bacc
from concourse import mybir
from concourse import bass_utils
from concourse.masks import make_identity

P = 128          # partitions
EMB = 256        # head dim
S = 4096         # sequence length
B = 8            # batch == number of cores
QB = 1024        # q-block
HB = 512         # q-half (one PSUM bank of fp32)

F32 = mybir.dt.float32
BF16 = mybir.dt.bfloat16
AF = mybir.ActivationFunctionType

X_BATCHES = (4, 4, 8, 8, 8)   # 128-row x tiles per input DMA


def _build(nc: bass.Bass, s_len: int = S) -> None:
    """Emit the per-core program into `nc` (SPMD: same program all cores)."""
    x = nc.dram_tensor("x", (s_len, EMB), F32, kind="ExternalInput").ap()
    Wq = nc.dram_tensor("Wq", (EMB, EMB), F32, kind="ExternalInput").ap()
    bq = nc.dram_tensor("bq", (EMB,), F32, kind="ExternalInput").ap()
    Wk = nc.dram_tensor("Wk", (EMB, EMB), F32, kind="ExternalInput").ap()
    Wv = nc.dram_tensor("Wv", (EMB, EMB), F32, kind="ExternalInput").ap()
    bv = nc.dram_tensor("bv", (EMB,), F32, kind="ExternalInput").ap()
    out = nc.dram_tensor("out", (s_len, EMB), F32, kind="ExternalOutput").ap()

    n_st = s_len // P      # 128-row tiles of the sequence
    n_qb = s_len // QB     # q-blocks
    n_kt = s_len // P      # k-tiles
    n_qt = QB // P         # 128-row q-tiles per q-block
    scale = float(EMB) ** -0.5

    with tile.TileContext(nc) as tc, ExitStack() as ctx:
        consts = ctx.enter_context(tc.tile_pool(name="consts", bufs=1))
        persist = ctx.enter_context(tc.tile_pool(name="persist", bufs=1))
        stage = ctx.enter_context(tc.tile_pool(name="stage", bufs=2))
        work = ctx.enter_context(tc.tile_pool(name="work", bufs=2))
        ps = ctx.enter_context(tc.tile_pool(name="ps", bufs=2, space="PSUM"))

        # ---- constants (no DMA deps: ready before the first transpose) ----
        idf = consts.tile([P, P], F32)
        make_identity(nc, idf)
        idb = consts.tile([P, P], BF16)
        nc.vector.tensor_copy(idb, idf)
        ones_f = consts.tile([P, 1], F32)
        nc.vector.memset(ones_f, 1.0)
        ones_bf = consts.tile([P, 1], BF16)
        nc.vector.memset(ones_bf, 1.0)

        # ---- input DMAs: everything else hides under them ----
        # order matters: HWDGE desc-gen and the DMA engines serialize; the
        # first PE work is x0 transposes, then A = f(Wq, Wk).
        xst = []
        t0 = 0

        def dma_x_batch(bi):
            nonlocal t0
            nb = X_BATCHES[bi]
            xb = stage.tile([P, nb, EMB], F32, tag="xst", name=f"xst{bi}")
            src = bass.AP(
                tensor=x.tensor, offset=x.offset + t0 * P * EMB,
                ap=[[EMB, P], [P * EMB, nb], [1, EMB]])
            nc.sync.dma_start(xb, src)
            xst.append(xb)
            t0 += nb

        dma_x_batch(0)
        bq_row = consts.tile([1, EMB], F32)
        nc.sync.dma_start(bq_row, bass.AP(tensor=bq.tensor, offset=bq.offset,
                                          ap=[[0, 1], list(bq.ap[0])]))
        wq_st = stage.tile([P, 2, EMB], F32, tag="wst", bufs=3, name="wq_st")
        nc.sync.dma_start(wq_st, Wq.rearrange("(t p) m -> p t m", p=P))
        wk_st = stage.tile([P, 2, EMB], F32, tag="wst", bufs=3, name="wk_st")
        nc.sync.dma_start(wk_st, Wk.rearrange("(t p) m -> p t m", p=P))
        wv_st = stage.tile([P, 2, EMB], F32, tag="wst", bufs=3, name="wv_st")
        nc.sync.dma_start(wv_st, Wv.rearrange("(t p) m -> p t m", p=P))
        dma_x_batch(1)
        dma_x_batch(2)
        bv_bc = consts.tile([P, EMB], F32)
        nc.sync.dma_start(
            bv_bc,
            bass.AP(tensor=bv.tensor, offset=bv.offset, ap=[[0, P], list(bv.ap[0])]),
        )
        for bi in range(3, len(X_BATCHES)):
            dma_x_batch(bi)

        # ---- weights: A = Wq^T @ Wk, u = bq @ Wk, WvT ----
        # Wq on DVE, Wk on Act: the casts run in parallel so A starts earliest
        wq_bf = persist.tile([P, 2, EMB], BF16)
        nc.vector.tensor_copy(wq_bf, wq_st)
        wk_bf = persist.tile([P, 2, EMB], BF16)
        nc.scalar.copy(wk_bf, wk_st)
        wv_bf = persist.tile([P, 2, EMB], BF16)
        nc.scalar.copy(wv_bf, wv_st)
        bq_bf = consts.tile([P, 2], BF16)
        for ec in range(2):
            btp = ps.tile([P, 1], F32, tag="sc", name=f"btp{ec}")
            nc.tensor.transpose(btp, bq_row[0:1, ec * P:(ec + 1) * P],
                                ones_f[0:1, 0:1])
            nc.scalar.copy(bq_bf[:, ec:ec + 1], btp)

        A_sb = persist.tile([P, 2, EMB], BF16)
        for dc in range(2):
            aps = ps.tile([P, EMB], F32, tag="sc", name=f"aps{dc}")
            for ec in range(2):
                nc.tensor.matmul(aps, wq_bf[:, ec, dc * P:(dc + 1) * P],
                                 wk_bf[:, ec, :], start=(ec == 0), stop=(ec == 1))
            nc.scalar.copy(A_sb[:, dc, :], aps)

        u_ps = ps.tile([1, EMB], F32, tag="sc")
        for ec in range(2):
            nc.tensor.matmul(u_ps, bq_bf[:, ec:ec + 1], wk_bf[:, ec, :],
                             start=(ec == 0), stop=(ec == 1))
        u_sb = work.tile([1, EMB], F32, tag="u_sb")
        nc.scalar.copy(u_sb, u_ps)
        u_col = consts.tile([P, 2], F32)
        for jc in range(2):
            utp = ps.tile([P, 1], F32, tag="sc", name=f"utp{jc}")
            nc.tensor.transpose(utp, u_sb[0:1, jc * P:(jc + 1) * P],
                                ones_f[0:1, 0:1])
            nc.scalar.copy(u_col[:, jc:jc + 1], utp)

        WvT = persist.tile([P, 2, EMB], BF16)
        for dc in range(2):
            for et in range(2):
                tp = ps.tile([P, P], BF16, tag="sc", name=f"wvtp{dc}{et}")
                nc.tensor.transpose(tp, wv_bf[:, et, dc * P:(dc + 1) * P], idb)
                nc.scalar.copy(WvT[:, dc, et * P:(et + 1) * P], tp)

        # ---- x: cast, PE-transpose -> xT[d, s]; project V and Q' ----
        # 4-tile groups share one PSUM tile per stage (transposes, V, Q') so
        # the 2-slot PSUM rotation amortizes the cross-engine copy latency;
        # V(g) and Q'(g) trail the transposes of group g+1.
        xT = persist.tile([P, 2, s_len], BF16, name="xT")
        QpT = persist.tile([P, 2, s_len], BF16, name="QpT")
        Vb = persist.tile([P, n_st, EMB], BF16, name="Vb")
        GT = 4   # tiles per group

        def emit_vqp(g):
            gsl = slice(g * GT * P, (g + 1) * GT * P)
            vB = ps.tile([P, GT, EMB], F32, tag="sc", name=f"vB{g}")
            for tl in range(GT):
                tsl = slice((g * GT + tl) * P, (g * GT + tl + 1) * P)
                for dc in range(2):
                    nc.tensor.matmul(vB[:, tl, :], xT[:, dc, tsl], WvT[:, dc, :],
                                     start=(dc == 0), stop=(dc == 1))
            nc.vector.tensor_copy(Vb[:, g * GT:(g + 1) * GT, :], vB)
            qpB = ps.tile([P, 2, HB], F32, tag="sc", name=f"qpB{g}")
            for jc in range(2):
                for dc in range(2):
                    nc.tensor.matmul(qpB[:, jc, :], A_sb[:, dc, jc * P:(jc + 1) * P],
                                     xT[:, dc, gsl],
                                     start=(dc == 0), stop=(dc == 1))
            for jc in range(2):
                nc.scalar.activation(QpT[:, jc, gsl], qpB[:, jc, :], AF.Identity,
                                     bias=u_col[:, jc:jc + 1], scale=1.0)

        g_i = 0
        for bi, nb in enumerate(X_BATCHES):
            xbf = stage.tile([P, nb, EMB], BF16, tag="xbf", name=f"xbf{bi}")
            nc.gpsimd.tensor_copy(xbf, xst[bi])
            for t0g in range(0, nb, GT):
                gsl = slice(g_i * GT * P, (g_i + 1) * GT * P)
                tpB = ps.tile([P, 2, GT * P], BF16, tag="sc", name=f"tpB{g_i}")
                for tl in range(GT):
                    for dc in range(2):
                        nc.tensor.transpose(
                            tpB[:, dc, tl * P:(tl + 1) * P],
                            xbf[:, t0g + tl, dc * P:(dc + 1) * P], idb)
                nc.vector.tensor_copy(xT[:, :, gsl], tpB)
                if g_i >= 2:
                    emit_vqp(g_i - 2)
                g_i += 1
        emit_vqp(g_i - 2)
        emit_vqp(g_i - 1)

        # ---- attention ----
        # q-blocks of (start, n_half) in 512-wide halves; the narrower final
        # blocks shorten the end-of-kernel drain (PV lag + finalize chain).
        # Per k-tile: ONE [128, n_h*512] PSUM score tile (bank per half), ONE
        # exp, ONE DVE denominator accumulate - minimizes the per-instruction
        # semaphore-wait overhead on the PE stream.
        qblocks = [(0, 2), (1024, 2), (2048, 2), (3072, 1), (3584, 1)]
        LAG = 2
        for qb_i, (q0b, n_h) in enumerate(qblocks):
            nq = n_h * 4   # 128-row q-tiles in this block
            out_ps = ps.tile([P, 8, EMB], F32, tag="po", bufs=1,
                             name=f"out_ps_{qb_i}")
            # two interleaved denominator accumulators (DVE + gpsimd) so
            # neither chain lags the PE and holds exp buffers alive
            dacc = [work.tile([P, n_h, HB], F32, tag=f"dacc{i}", bufs=2,
                              name=f"dacc{i}_{qb_i}") for i in range(2)]
            elist = []

            def emit_pv(kp):
                for h in range(n_h):
                    for j in range(4):
                        jg = h * 4 + j
                        nc.tensor.matmul(out_ps[:, jg, :],
                                         elist[kp][:, h, j * P:(j + 1) * P],
                                         Vb[:, kp, :],
                                         start=(kp == 0 and jg % 2 == 0),
                                         stop=(kp == n_kt - 1 and jg % 2 == 1))

            for kt_i in range(n_kt):
                ksl = slice(kt_i * P, (kt_i + 1) * P)
                sc = ps.tile([P, n_h, HB], F32, tag="sc",
                             name=f"sc{qb_i}_{kt_i}")
                for dc in range(2):   # lhsT reused across halves: 1 LDWEIGHTS
                    for h in range(n_h):
                        hsl = slice(q0b + h * HB, q0b + (h + 1) * HB)
                        nc.tensor.matmul(sc[:, h, :], xT[:, dc, ksl],
                                         QpT[:, dc, hsl],
                                         start=(dc == 0), stop=(dc == 1))
                ebf = work.tile([P, n_h, HB], BF16, tag="E", bufs=10,
                                name=f"e{qb_i}_{kt_i}")
                nc.scalar.activation(ebf, sc, AF.Exp, scale=scale)
                if kt_i < n_kt - 1:   # last tile's sum comes straight from ebf
                    ci = kt_i % 2
                    eng = nc.vector if ci == 0 else nc.gpsimd
                    da = dacc[ci]
                    if kt_i < 2:
                        eng.tensor_copy(da, ebf)
                    else:
                        eng.tensor_add(da, da, ebf)
                elist.append(ebf)
                if kt_i >= LAG:
                    emit_pv(kt_i - LAG)
            for kp in range(n_kt - LAG, n_kt):
                emit_pv(kp)

            # denominators: tiny N=1 matmuls chunk.T @ ones -> [q, 1] columns
            # in one PSUM bank (an "sc" slot, free during the boundary). The
            # last k-tile's term reads the exp tile directly so the chain
            # tails don't gate the finalize.
            dn_ps = ps.tile([P, nq], F32, tag="sc", name=f"dn_{qb_i}")
            srcs = [(dacc[0], ones_f), (dacc[1], ones_f), (elist[-1], ones_bf)]
            for si, (dsrc, drhs) in enumerate(srcs):
                for j in range(nq):
                    nc.tensor.matmul(
                        dn_ps[:, j:j + 1],
                        dsrc[:, j // 4, (j % 4) * P:(j % 4 + 1) * P], drhs,
                        start=(si == 0 and j == 0),
                        stop=(si == 2 and j == nq - 1))
            recip = work.tile([P, 8], F32, tag="recip", name=f"recip{qb_i}")
            nc.vector.reciprocal(recip[:, 0:nq], dn_ps)
            ost = work.tile([P, 8, EMB], F32, tag="ost", name=f"ost{qb_i}")
            for j in range(nq):
                nc.vector.scalar_tensor_tensor(
                    ost[:, j, :], out_ps[:, j, :], recip[:, j:j + 1], bv_bc,
                    op0=mybir.AluOpType.mult, op1=mybir.AluOpType.add)
            for half in range(n_h):
                q0 = q0b + half * HB
                dst = bass.AP(
                    tensor=out.tensor, offset=out.offset + q0 * EMB,
                    ap=[[EMB, P], [P * EMB, 4], [1, EMB]])
                nc.sync.dma_start(dst, ost[:, half * 4:(half + 1) * 4, :])


def _make_nc(s_len: int = S) -> bass.Bass:
    # Bacc (not raw Bass): its compile() splits multi-sem waits and moves
    # matmul waits onto ldweights - HW allows at most one wait per inst.
    nc = bacc.Bacc("TRN2", target_bir_lowering=False, debug=False)
    _build(nc, s_len)
    nc.compile()
    return nc


def _prep(inputs: dict) -> dict:
    arrs = {k: np.ascontiguousarray(np.asarray(v, dtype=np.float32))
            for k, v in inputs.items()}
    assert arrs["x"].shape == (B, S, EMB), arrs["x"].shape
    return arrs


def run(inputs: dict):
    """Run on 8 NeuronCores. Returns (out[B,S,E] f32, BassKernelResults)."""
    arrs = _prep(inputs)
    nc = _make_nc(S)
    shared = {k: arrs[k] for k in ("Wq", "bq", "Wk", "Wv", "bv")}
    in_maps = [dict(shared, x=arrs["x"][i]) for i in range(B)]
    res = bass_utils.run_bass_kernel_spmd(nc, in_maps, core_ids=list(range(B)))
    out = np.stack([r["out"] for r in res.results], axis=0).astype(np.float32)
    return out, res


def kernel(**inputs) -> np.ndarray:
    out, _ = run(inputs)
    return out


def bench(inputs: dict, iters: int = 5, chain: int = 1):
    """Compile once, then time repeated executions with device-resident
    inputs (mirrors bass2jax.run_bass_via_pjrt's multi-core path).

    `chain` > 1 executes the NEFF that many times inside one XLA program
    (each call's outputs feed the next call's donated output buffers, which
    serializes them) so per-iteration device time can be extracted as a
    slope, amortizing the axon dispatch overhead.

    Returns (out[B,S,E] f32, list of per-call wall times in seconds).
    """
    import time

    import jax
    from jax.sharding import Mesh, NamedSharding, PartitionSpec
    from jax.experimental.shard_map import shard_map

    from concourse import bass2jax
    from concourse import mybir as mb

    arrs = _prep(inputs)
    nc = _make_nc(S)
    bass2jax.install_neuronx_cc_hook()

    partition_name = (
        nc.partition_id_tensor.name if nc.partition_id_tensor else None
    )
    in_names, out_names, out_avals, zero_outs = [], [], [], []
    for alloc in nc.m.functions[0].allocations:
        if not isinstance(alloc, mb.MemoryLocationSet):
            continue
        name = alloc.memorylocations[0].name
        if alloc.kind == "ExternalInput":
            if name != partition_name:
                in_names.append(name)
        elif alloc.kind == "ExternalOutput":
            out_names.append(name)
            shape = tuple(alloc.tensor_shape)
            dtype = mb.dt.np(alloc.dtype)
            out_avals.append(jax.core.ShapedArray(shape, dtype))
            zero_outs.append(np.zeros(shape, dtype))
    n_params = len(in_names)
    n_outs = len(out_avals)
    all_names = in_names + out_names
    if partition_name is not None:
        all_names = all_names + [partition_name]

    def _call(ins, zeros):
        operands = list(ins) + list(zeros)
        if partition_name is not None:
            operands.append(bass2jax.partition_id_tensor())
        return bass2jax._bass_exec_p.bind(
            *operands,
            out_avals=tuple(out_avals),
            in_names=tuple(all_names),
            out_names=tuple(out_names),
            lowering_input_output_aliases=(),
            sim_require_finite=True,
            sim_require_nnan=True,
            nc=nc,
        )

    def _body(*args):
        ins = list(args[:n_params])
        zeros = list(args[n_params:])
        outs = _call(ins, zeros)
        for _ in range(chain - 1):
            outs = _call(ins, list(outs))
        return tuple(outs)

    devices = jax.devices()[:B]
    mesh = Mesh(np.asarray(devices), ("core",))
    in_specs = (PartitionSpec("core"),) * (n_params + n_outs)
    out_specs = (PartitionSpec("core"),) * n_outs
    donate = tuple(range(n_params, n_params + n_outs))
    sharded = jax.jit(
        shard_map(_body, mesh=mesh, in_specs=in_specs, out_specs=out_specs,
                  check_rep=False),
        donate_argnums=donate,
        keep_unused=True,
    )

    per_core = [
        [arrs["x"][c] if n == "x" else arrs[n] for n in in_names[:n_params]]
        for c in range(B)
    ]
    concat_in = [
        np.concatenate([per_core[c][i] for c in range(B)], axis=0)
        for i in range(n_params)
    ]
    concat_zeros = [
        np.zeros((B * z.shape[0], *z.shape[1:]), z.dtype) for z in zero_outs
    ]

    shard = NamedSharding(mesh, PartitionSpec("core"))
    dev_in = [jax.device_put(a, shard) for a in concat_in]
    jax.block_until_ready(dev_in)

    times = []
    out_np = None
    for i in range(iters + 1):
        dev_zeros = [jax.device_put(z, shard) for z in concat_zeros]
        jax.block_until_ready(dev_zeros)
        t0 = time.perf_counter()
        outs = sharded(*dev_in, *dev_zeros)
        jax.block_until_ready(outs)
        dt = time.perf_counter() - t0
        if i == 0:
            idx = out_names.index("out")
            out_np = np.asarray(outs[idx]).reshape(B, S, EMB).astype(np.float32)
        else:
            times.append(dt)
    return out_np, times


# revision 21
# speedup vs baseline: 1.1454x; 1.0380x over previous
"""Single-head MHA (QKV proj + softmax attention) on 8 Trainium2 cores.

Problem: x[8, 4096, 256] f32; per-batch attention with per-head emb 256.
Sharding: data-parallel - one batch element per NeuronCore (8 cores).

Per-core algorithm (S=4096, E=256, P=128 partitions), all matmuls bf16:
  - A = Wq^T @ Wk [256, 256] once (tiny), so scores = (x @ A) @ x^T and the
    K projection disappears; the bq bias folds in exactly as a per-partition
    column u = bq @ Wk on the Q' projection, and the bk bias term is
    constant per q-row so it cancels in softmax.
  - x arrives in 5 batched DMAs; per 128-row tile: cast to bf16 (gpsimd),
    PE-transpose into xT[d, s], V-tile = xT.T @ WvT, and per 512 columns
    Q'T[e', s] = A.T @ xT (+u bias fused in the PSUM->SBUF copy).
  - attention per q-block of 1024 columns, two 512-wide halves per k-tile:
      S^T[k, qh] = xT_slice.T @ Q'T   (2 matmuls, fp32 PSUM, 1-bank tiles)
      E[k, qh]   = exp(S^T / 16)      (ScalarE, scale fused, bf16 out)
      out[q, e] += E_chunk.T @ V      (4 matmuls N=256 per half, lagged 4
                                       k-tiles; E q-chunks stationary so the
                                       output lands in [q, e] - no transposes)
      dn[q]     += E_chunk.T @ ones   (4 tiny N=1 matmuls per half into a
                                       dedicated PSUM bank: the softmax
                                       denominator costs no DVE time and is
                                       complete the moment the last exp is)
    finalize: recip (DVE), then out = out_ps*recip + bv per 128-row tile
    (softmax rows sum to 1, so attn @ (V + bv) = attn @ V + bv), alternating
    DVE / gpsimd, written to a staging tile and DMA'd out in 512-row blocks.
    No PE instruction depends on the finalize, so the PE streams straight
    into the next q-block.

PSUM budget exactly 8 banks: 3x[128,512]f32 score slots + [128,8,256]f32
PV accumulator (4 banks) + [128,8]f32 denominator bank.

No running-max subtraction: scores/16 ~ N(0,1); max observed ~10.5, exp
stays well inside fp32/bf16 range.
"""

from contextlib import ExitStack

import numpy as np

import concourse.bass as bass
import concourse.tile as tile
from concourse import bacc
from concourse import mybir
from concourse import bass_utils
from concourse.masks import make_identity

P = 128          # partitions
EMB = 256        # head dim
S = 4096         # sequence length
B = 8            # batch == number of cores
QB = 1024        # q-block
HB = 512         # q-half (one PSUM bank of fp32)

F32 = mybir.dt.float32
BF16 = mybir.dt.bfloat16
AF = mybir.ActivationFunctionType

X_BATCHES = (4,) * 8   # 128-row x tiles per input DMA


def _build(nc: bass.Bass, s_len: int = S) -> None:
    """Emit the per-core program into `nc` (SPMD: same program all cores)."""
    x = nc.dram_tensor("x", (s_len, EMB), F32, kind="ExternalInput").ap()
    Wq = nc.dram_tensor("Wq", (EMB, EMB), F32, kind="ExternalInput").ap()
    bq = nc.dram_tensor("bq", (EMB,), F32, kind="ExternalInput").ap()
    Wk = nc.dram_tensor("Wk", (EMB, EMB), F32, kind="ExternalInput").ap()
    Wv = nc.dram_tensor("Wv", (EMB, EMB), F32, kind="ExternalInput").ap()
    bv = nc.dram_tensor("bv", (EMB,), F32, kind="ExternalInput").ap()
    out = nc.dram_tensor("out", (s_len, EMB), F32, kind="ExternalOutput").ap()

    n_st = s_len // P      # 128-row tiles of the sequence
    n_qb = s_len // QB     # q-blocks
    n_kt = s_len // P      # k-tiles
    n_qt = QB // P         # 128-row q-tiles per q-block
    scale = float(EMB) ** -0.5

    with tile.TileContext(nc) as tc, ExitStack() as ctx:
        consts = ctx.enter_context(tc.tile_pool(name="consts", bufs=1))
        persist = ctx.enter_context(tc.tile_pool(name="persist", bufs=1))
        stage = ctx.enter_context(tc.tile_pool(name="stage", bufs=2))
        work = ctx.enter_context(tc.tile_pool(name="work", bufs=2))
        ps = ctx.enter_context(tc.tile_pool(name="ps", bufs=2, space="PSUM"))

        # ---- constants (no DMA deps: ready before the first transpose) ----
        idf = consts.tile([P, P], F32)
        make_identity(nc, idf)
        idb = consts.tile([P, P], BF16)
        nc.vector.tensor_copy(idb, idf)
        ones_f = consts.tile([P, 1], F32)
        nc.vector.memset(ones_f, 1.0)
        ones_bf = consts.tile([P, 1], BF16)
        nc.vector.memset(ones_bf, 1.0)

        # ---- input DMAs: everything else hides under them ----
        # order matters: HWDGE desc-gen and the DMA engines serialize; the
        # first PE work is x0 transposes, then A = f(Wq, Wk).
        xst = []
        t0 = 0

        def dma_x_batch(bi):
            nonlocal t0
            nb = X_BATCHES[bi]
            xb = stage.tile([P, nb, EMB], F32, tag="xst", name=f"xst{bi}")
            src = bass.AP(
                tensor=x.tensor, offset=x.offset + t0 * P * EMB,
                ap=[[EMB, P], [P * EMB, nb], [1, EMB]])
            nc.sync.dma_start(xb, src)
            xst.append(xb)
            t0 += nb

        dma_x_batch(0)
        bq_row = consts.tile([1, EMB], F32)
        nc.sync.dma_start(bq_row, bass.AP(tensor=bq.tensor, offset=bq.offset,
                                          ap=[[0, 1], list(bq.ap[0])]))
        wq_st = stage.tile([P, 2, EMB], F32, tag="wst", bufs=3, name="wq_st")
        nc.sync.dma_start(wq_st, Wq.rearrange("(t p) m -> p t m", p=P))
        wk_st = stage.tile([P, 2, EMB], F32, tag="wst", bufs=3, name="wk_st")
        nc.sync.dma_start(wk_st, Wk.rearrange("(t p) m -> p t m", p=P))
        dma_x_batch(1)
        wv_st = stage.tile([P, 2, EMB], F32, tag="wst", bufs=3, name="wv_st")
        nc.sync.dma_start(wv_st, Wv.rearrange("(t p) m -> p t m", p=P))
        dma_x_batch(2)
        dma_x_batch(3)
        bv_bc = consts.tile([P, EMB], F32)
        nc.sync.dma_start(
            bv_bc,
            bass.AP(tensor=bv.tensor, offset=bv.offset, ap=[[0, P], list(bv.ap[0])]),
        )
        for bi in range(4, len(X_BATCHES)):
            dma_x_batch(bi)

        # ---- weights: A = Wq^T @ Wk, u = bq @ Wk, WvT ----
        # Wq on DVE, Wk on Act: the casts run in parallel so A starts earliest
        wq_bf = persist.tile([P, 2, EMB], BF16)
        nc.vector.tensor_copy(wq_bf, wq_st)
        wk_bf = persist.tile([P, 2, EMB], BF16)
        nc.scalar.copy(wk_bf, wk_st)
        wv_bf = persist.tile([P, 2, EMB], BF16)
        nc.scalar.copy(wv_bf, wv_st)
        bq_bf = consts.tile([P, 2], BF16)
        for ec in range(2):
            btp = ps.tile([P, 1], F32, tag="sc", name=f"btp{ec}")
            nc.tensor.transpose(btp, bq_row[0:1, ec * P:(ec + 1) * P],
                                ones_f[0:1, 0:1])
            nc.vector.tensor_copy(bq_bf[:, ec:ec + 1], btp)

        A_sb = persist.tile([P, 2, EMB], BF16)
        WvT = persist.tile([P, 2, EMB], BF16)
        u_col = consts.tile([P, 2], F32)

        def emit_weights():
            for dc in range(2):
                aps = ps.tile([P, EMB], F32, tag="sc", name=f"aps{dc}")
                for ec in range(2):
                    nc.tensor.matmul(aps, wq_bf[:, ec, dc * P:(dc + 1) * P],
                                     wk_bf[:, ec, :],
                                     start=(ec == 0), stop=(ec == 1))
                nc.vector.tensor_copy(A_sb[:, dc, :], aps)
            u_ps = ps.tile([1, EMB], F32, tag="sc")
            for ec in range(2):
                nc.tensor.matmul(u_ps, bq_bf[:, ec:ec + 1], wk_bf[:, ec, :],
                                 start=(ec == 0), stop=(ec == 1))
            u_sb = work.tile([1, EMB], F32, tag="u_sb")
            nc.vector.tensor_copy(u_sb, u_ps)
            for jc in range(2):
                utp = ps.tile([P, 1], F32, tag="sc", name=f"utp{jc}")
                nc.tensor.transpose(utp, u_sb[0:1, jc * P:(jc + 1) * P],
                                    ones_f[0:1, 0:1])
                nc.vector.tensor_copy(u_col[:, jc:jc + 1], utp)
            for dc in range(2):
                for et in range(2):
                    tp = ps.tile([P, P], BF16, tag="sc", name=f"wvtp{dc}{et}")
                    nc.tensor.transpose(tp, wv_bf[:, et, dc * P:(dc + 1) * P],
                                        idb)
                    nc.vector.tensor_copy(WvT[:, dc, et * P:(et + 1) * P], tp)

        # ---- x: cast, PE-transpose -> xT[d, s]; project V and Q' ----
        # 4-tile groups share one PSUM tile per stage (transposes, V, Q') so
        # the 2-slot PSUM rotation amortizes the cross-engine copy latency;
        # V(g) and Q'(g) trail the transposes of group g+1.
        xT = persist.tile([P, 2, s_len], BF16, name="xT")
        QpT = persist.tile([P, 2, s_len], BF16, name="QpT")
        Vb = persist.tile([P, n_st, EMB], BF16, name="Vb")
        GT = 4   # tiles per group

        def emit_vqp(g):
            gsl = slice(g * GT * P, (g + 1) * GT * P)
            # the PV accumulator bank-group is idle during the front: use
            # it for the V-projection batches so the "sc" rotation only has
            # to cycle the transpose and Q' tiles
            vB = ps.tile([P, GT, EMB], F32, tag="po", bufs=1, name=f"vB{g}")
            for tl in range(GT):
                tsl = slice((g * GT + tl) * P, (g * GT + tl + 1) * P)
                for dc in range(2):
                    nc.tensor.matmul(vB[:, tl, :], xT[:, dc, tsl], WvT[:, dc, :],
                                     start=(dc == 0), stop=(dc == 1))
            nc.vector.tensor_copy(Vb[:, g * GT:(g + 1) * GT, :], vB)
            qpB = ps.tile([P, 2, HB], F32, tag="sc", name=f"qpB{g}")
            for jc in range(2):
                for dc in range(2):
                    nc.tensor.matmul(qpB[:, jc, :], A_sb[:, dc, jc * P:(jc + 1) * P],
                                     xT[:, dc, gsl],
                                     start=(dc == 0), stop=(dc == 1))
            for jc in range(2):
                nc.scalar.activation(QpT[:, jc, gsl], qpB[:, jc, :], AF.Identity,
                                     bias=u_col[:, jc:jc + 1], scale=1.0)

        g_i = 0
        for bi, nb in enumerate(X_BATCHES):
            xbf = stage.tile([P, nb, EMB], BF16, tag="xbf", name=f"xbf{bi}")
            nc.gpsimd.tensor_copy(xbf, xst[bi])
            for t0g in range(0, nb, GT):
                gsl = slice(g_i * GT * P, (g_i + 1) * GT * P)
                tpB = ps.tile([P, 2, GT * P], BF16, tag="sc", name=f"tpB{g_i}")
                for tl in range(GT):
                    for dc in range(2):
                        nc.tensor.transpose(
                            tpB[:, dc, tl * P:(tl + 1) * P],
                            xbf[:, t0g + tl, dc * P:(dc + 1) * P], idb)
                nc.vector.tensor_copy(xT[:, :, gsl], tpB)
                if g_i == 0:
                    emit_weights()   # fills the PE while x batch 1 lands
                if g_i >= 2:
                    emit_vqp(g_i - 2)
                g_i += 1
        emit_vqp(g_i - 2)
        emit_vqp(g_i - 1)

        # ---- attention ----
        # q-blocks of (start, n_half) in 512-wide halves; the narrower final
        # blocks shorten the end-of-kernel drain (PV lag + finalize chain).
        # Per k-tile: ONE [128, n_h*512] PSUM score tile (bank per half), ONE
        # exp, ONE DVE denominator accumulate - minimizes the per-instruction
        # semaphore-wait overhead on the PE stream.
        qblocks = [(0, 2), (1024, 2), (2048, 2), (3072, 1), (3584, 1)]
        for qb_i, (q0b, n_h) in enumerate(qblocks):
            nq = n_h * 4   # 128-row q-tiles in this block
            # deep lag mid-kernel so the next block's PV start always lands
            # after this block's finalize; shallow on the last block so the
            # end-of-kernel drain is short
            LAG = 2 if qb_i == len(qblocks) - 1 else 4
            out_ps = ps.tile([P, 8, EMB], F32, tag="po", bufs=1,
                             name=f"out_ps_{qb_i}")
            # two interleaved denominator accumulators (DVE + gpsimd) so
            # neither chain lags the PE and holds exp buffers alive
            dacc = [work.tile([P, n_h, HB], F32, tag=f"dacc{i}", bufs=2,
                              name=f"dacc{i}_{qb_i}") for i in range(2)]
            elist = []

            def emit_pv(kp):
                for h in range(n_h):
                    for j in range(4):
                        jg = h * 4 + j
                        nc.tensor.matmul(out_ps[:, jg, :],
                                         elist[kp][:, h, j * P:(j + 1) * P],
                                         Vb[:, kp, :],
                                         start=(kp == 0 and jg % 2 == 0),
                                         stop=(kp == n_kt - 1 and jg % 2 == 1))

            for kt_i in range(n_kt):
                ksl = slice(kt_i * P, (kt_i + 1) * P)
                sc = ps.tile([P, n_h, HB], F32, tag="sc",
                             name=f"sc{qb_i}_{kt_i}")
                for dc in range(2):   # lhsT reused across halves: 1 LDWEIGHTS
                    for h in range(n_h):
                        hsl = slice(q0b + h * HB, q0b + (h + 1) * HB)
                        nc.tensor.matmul(sc[:, h, :], xT[:, dc, ksl],
                                         QpT[:, dc, hsl],
                                         start=(dc == 0), stop=(dc == 1))
                ebf = work.tile([P, n_h, HB], BF16, tag="E", bufs=10,
                                name=f"e{qb_i}_{kt_i}")
                nc.scalar.activation(ebf, sc, AF.Exp, scale=scale)
                if kt_i < n_kt - 1:   # last tile's sum comes straight from ebf
                    ci = kt_i % 2
                    eng = nc.vector if ci == 0 else nc.gpsimd
                    da = dacc[ci]
                    if kt_i < 2:
                        eng.tensor_copy(da, ebf)
                    else:
                        eng.tensor_add(da, da, ebf)
                elist.append(ebf)
                if kt_i >= LAG:
                    emit_pv(kt_i - LAG)
            for kp in range(n_kt - LAG, n_kt):
                emit_pv(kp)

            # denominators: tiny N=1 matmuls chunk.T @ ones -> [q, 1] columns
            # in one PSUM bank (an "sc" slot, free during the boundary). The
            # last k-tile's term reads the exp tile directly so the chain
            # tails don't gate the finalize.
            dn_ps = ps.tile([P, nq], F32, tag="sc", name=f"dn_{qb_i}")
            srcs = [(dacc[0], ones_f), (dacc[1], ones_f), (elist[-1], ones_bf)]
            for si, (dsrc, drhs) in enumerate(srcs):
                for j in range(nq):
                    nc.tensor.matmul(
                        dn_ps[:, j:j + 1],
                        dsrc[:, j // 4, (j % 4) * P:(j % 4 + 1) * P], drhs,
                        start=(si == 0 and j == 0),
                        stop=(si == 2 and j == nq - 1))
            recip = work.tile([P, 8], F32, tag="recip", name=f"recip{qb_i}")
            nc.vector.reciprocal(recip[:, 0:nq], dn_ps)
            ost = work.tile([P, 8, EMB], F32, tag="ost", name=f"ost{qb_i}")
            for j in range(nq):
                nc.vector.scalar_tensor_tensor(
                    ost[:, j, :], out_ps[:, j, :], recip[:, j:j + 1], bv_bc,
                    op0=mybir.AluOpType.mult, op1=mybir.AluOpType.add)
            last = qb_i == len(qblocks) - 1
            chunk = 1 if last else 2
            for ci in range(nq // chunk):
                q0 = q0b + ci * chunk * P
                dst = bass.AP(
                    tensor=out.tensor, offset=out.offset + q0 * EMB,
                    ap=[[EMB, P], [P * EMB, chunk], [1, EMB]])
                nc.sync.dma_start(dst, ost[:, ci * chunk:(ci + 1) * chunk, :])


def _make_nc(s_len: int = S) -> bass.Bass:
    # Bacc (not raw Bass): its compile() splits multi-sem waits and moves
    # matmul waits onto ldweights - HW allows at most one wait per inst.
    nc = bacc.Bacc("TRN2", target_bir_lowering=False, debug=False)
    _build(nc, s_len)
    nc.compile()
    return nc


def _prep(inputs: dict) -> dict:
    arrs = {k: np.ascontiguousarray(np.asarray(v, dtype=np.float32))
            for k, v in inputs.items()}
    assert arrs["x"].shape == (B, S, EMB), arrs["x"].shape
    return arrs


def run(inputs: dict):
    """Run on 8 NeuronCores. Returns (out[B,S,E] f32, BassKernelResults)."""
    arrs = _prep(inputs)
    nc = _make_nc(S)
    shared = {k: arrs[k] for k in ("Wq", "bq", "Wk", "Wv", "bv")}
    in_maps = [dict(shared, x=arrs["x"][i]) for i in range(B)]
    res = bass_utils.run_bass_kernel_spmd(nc, in_maps, core_ids=list(range(B)))
    out = np.stack([r["out"] for r in res.results], axis=0).astype(np.float32)
    return out, res


def kernel(**inputs) -> np.ndarray:
    out, _ = run(inputs)
    return out


def bench(inputs: dict, iters: int = 5, chain: int = 1):
    """Compile once, then time repeated executions with device-resident
    inputs (mirrors bass2jax.run_bass_via_pjrt's multi-core path).

    `chain` > 1 executes the NEFF that many times inside one XLA program
    (each call's outputs feed the next call's donated output buffers, which
    serializes them) so per-iteration device time can be extracted as a
    slope, amortizing the axon dispatch overhead.

    Returns (out[B,S,E] f32, list of per-call wall times in seconds).
    """
    import time

    import jax
    from jax.sharding import Mesh, NamedSharding, PartitionSpec
    from jax.experimental.shard_map import shard_map

    from concourse import bass2jax
    from concourse import mybir as mb

    arrs = _prep(inputs)
    nc = _make_nc(S)
    bass2jax.install_neuronx_cc_hook()

    partition_name = (
        nc.partition_id_tensor.name if nc.partition_id_tensor else None
    )
    in_names, out_names, out_avals, zero_outs = [], [], [], []
    for alloc in nc.m.functions[0].allocations:
        if not isinstance(alloc, mb.MemoryLocationSet):
            continue
        name = alloc.memorylocations[0].name
        if alloc.kind == "ExternalInput":
            if name != partition_name:
                in_names.append(name)
        elif alloc.kind == "ExternalOutput":
            out_names.append(name)
            shape = tuple(alloc.tensor_shape)
            dtype = mb.dt.np(alloc.dtype)
            out_avals.append(jax.core.ShapedArray(shape, dtype))
            zero_outs.append(np.zeros(shape, dtype))
    n_params = len(in_names)
    n_outs = len(out_avals)
    all_names = in_names + out_names
    if partition_name is not None:
        all_names = all_names + [partition_name]

    def _call(ins, zeros):
        operands = list(ins) + list(zeros)
        if partition_name is not None:
            operands.append(bass2jax.partition_id_tensor())
        return bass2jax._bass_exec_p.bind(
            *operands,
            out_avals=tuple(out_avals),
            in_names=tuple(all_names),
            out_names=tuple(out_names),
            lowering_input_output_aliases=(),
            sim_require_finite=True,
            sim_require_nnan=True,
            nc=nc,
        )

    def _body(*args):
        ins = list(args[:n_params])
        zeros = list(args[n_params:])
        outs = _call(ins, zeros)
        for _ in range(chain - 1):
            outs = _call(ins, list(outs))
        return tuple(outs)

    devices = jax.devices()[:B]
    mesh = Mesh(np.asarray(devices), ("core",))
    in_specs = (PartitionSpec("core"),) * (n_params + n_outs)
    out_specs = (PartitionSpec("core"),) * n_outs
    donate = tuple(range(n_params, n_params + n_outs))
    sharded = jax.jit(
        shard_map(_body, mesh=mesh, in_specs=in_specs, out_specs=out_specs,
                  check_rep=False),
        donate_argnums=donate,
        keep_unused=True,
    )

    per_core = [
        [arrs["x"][c] if n == "x" else arrs[n] for n in in_names[:n_params]]
        for c in range(B)
    ]
    concat_in = [
        np.concatenate([per_core[c][i] for c in range(B)], axis=0)
        for i in range(n_params)
    ]
    concat_zeros = [
        np.zeros((B * z.shape[0], *z.shape[1:]), z.dtype) for z in zero_outs
    ]

    shard = NamedSharding(mesh, PartitionSpec("core"))
    dev_in = [jax.device_put(a, shard) for a in concat_in]
    jax.block_until_ready(dev_in)

    times = []
    out_np = None
    for i in range(iters + 1):
        dev_zeros = [jax.device_put(z, shard) for z in concat_zeros]
        jax.block_until_ready(dev_zeros)
        t0 = time.perf_counter()
        outs = sharded(*dev_in, *dev_zeros)
        jax.block_until_ready(outs)
        dt = time.perf_counter() - t0
        if i == 0:
            idx = out_names.index("out")
            out_np = np.asarray(outs[idx]).reshape(B, S, EMB).astype(np.float32)
        else:
            times.append(dt)
    return out_np, times


# revision 28
# speedup vs baseline: 1.1521x; 1.0059x over previous
"""Single-head MHA (QKV proj + softmax attention) on 8 Trainium2 cores.

Problem: x[8, 4096, 256] f32; per-batch attention with per-head emb 256.
Sharding: data-parallel - one batch element per NeuronCore (8 cores).

Per-core algorithm (S=4096, E=256, P=128 partitions), all matmuls bf16:
  - A = Wq^T @ Wk [256, 256] once (tiny), so scores = (x @ A) @ x^T and the
    K projection disappears; the bq bias folds in exactly as a per-partition
    column u = bq @ Wk on the Q' projection, and the bk bias term is
    constant per q-row so it cancels in softmax.
  - x arrives in 5 batched DMAs; per 128-row tile: cast to bf16 (gpsimd),
    PE-transpose into xT[d, s], V-tile = xT.T @ WvT, and per 512 columns
    Q'T[e', s] = A.T @ xT (+u bias fused in the PSUM->SBUF copy).
  - attention per q-block of 1024 columns, two 512-wide halves per k-tile:
      S^T[k, qh] = xT_slice.T @ Q'T   (2 matmuls, fp32 PSUM, 1-bank tiles)
      E[k, qh]   = exp(S^T / 16)      (ScalarE, scale fused, bf16 out)
      out[q, e] += E_chunk.T @ V      (4 matmuls N=256 per half, lagged 4
                                       k-tiles; E q-chunks stationary so the
                                       output lands in [q, e] - no transposes)
      dn[q]     += E_chunk.T @ ones   (4 tiny N=1 matmuls per half into a
                                       dedicated PSUM bank: the softmax
                                       denominator costs no DVE time and is
                                       complete the moment the last exp is)
    finalize: recip (DVE), then out = out_ps*recip + bv per 128-row tile
    (softmax rows sum to 1, so attn @ (V + bv) = attn @ V + bv), alternating
    DVE / gpsimd, written to a staging tile and DMA'd out in 512-row blocks.
    No PE instruction depends on the finalize, so the PE streams straight
    into the next q-block.

PSUM budget exactly 8 banks: 3x[128,512]f32 score slots + [128,8,256]f32
PV accumulator (4 banks) + [128,8]f32 denominator bank.

No running-max subtraction: scores/16 ~ N(0,1); max observed ~10.5, exp
stays well inside fp32/bf16 range.
"""

from contextlib import ExitStack

import numpy as np

import concourse.bass as bass
import concourse.tile as tile
from concourse import bacc
from concourse import mybir
from concourse import bass_utils
from concourse.masks import make_identity

P = 128          # partitions
EMB = 256        # head dim
S = 4096         # sequence length
B = 8            # batch == number of cores
QB = 1024        # q-block
HB = 512         # q-half (one PSUM bank of fp32)

F32 = mybir.dt.float32
BF16 = mybir.dt.bfloat16
AF = mybir.ActivationFunctionType

X_BATCHES = (4,) * 8   # 128-row x tiles per input DMA


def _build(nc: bass.Bass, s_len: int = S) -> None:
    """Emit the per-core program into `nc` (SPMD: same program all cores)."""
    x = nc.dram_tensor("x", (s_len, EMB), F32, kind="ExternalInput").ap()
    Wq = nc.dram_tensor("Wq", (EMB, EMB), F32, kind="ExternalInput").ap()
    bq = nc.dram_tensor("bq", (EMB,), F32, kind="ExternalInput").ap()
    Wk = nc.dram_tensor("Wk", (EMB, EMB), F32, kind="ExternalInput").ap()
    Wv = nc.dram_tensor("Wv", (EMB, EMB), F32, kind="ExternalInput").ap()
    bv = nc.dram_tensor("bv", (EMB,), F32, kind="ExternalInput").ap()
    out = nc.dram_tensor("out", (s_len, EMB), F32, kind="ExternalOutput").ap()

    n_st = s_len // P      # 128-row tiles of the sequence
    n_qb = s_len // QB     # q-blocks
    n_kt = s_len // P      # k-tiles
    n_qt = QB // P         # 128-row q-tiles per q-block
    scale = float(EMB) ** -0.5

    with tile.TileContext(nc) as tc, ExitStack() as ctx:
        consts = ctx.enter_context(tc.tile_pool(name="consts", bufs=1))
        persist = ctx.enter_context(tc.tile_pool(name="persist", bufs=1))
        stage = ctx.enter_context(tc.tile_pool(name="stage", bufs=2))
        work = ctx.enter_context(tc.tile_pool(name="work", bufs=2))
        ps = ctx.enter_context(tc.tile_pool(name="ps", bufs=2, space="PSUM"))

        # ---- constants (no DMA deps: ready before the first transpose) ----
        idf = consts.tile([P, P], F32)
        make_identity(nc, idf)
        idb = consts.tile([P, P], BF16)
        nc.vector.tensor_copy(idb, idf)
        ones_f = consts.tile([P, 1], F32)
        nc.vector.memset(ones_f, 1.0)
        ones_bf = consts.tile([P, 1], BF16)
        nc.vector.memset(ones_bf, 1.0)

        # ---- input DMAs: everything else hides under them ----
        # order matters: HWDGE desc-gen and the DMA engines serialize; the
        # first PE work is x0 transposes, then A = f(Wq, Wk).
        xst = []
        t0 = 0

        def dma_x_batch(bi):
            nonlocal t0
            nb = X_BATCHES[bi]
            xb = stage.tile([P, nb, EMB], F32, tag="xst", name=f"xst{bi}")
            src = bass.AP(
                tensor=x.tensor, offset=x.offset + t0 * P * EMB,
                ap=[[EMB, P], [P * EMB, nb], [1, EMB]])
            nc.sync.dma_start(xb, src)
            xst.append(xb)
            t0 += nb

        dma_x_batch(0)
        bq_row = consts.tile([1, EMB], F32)
        nc.sync.dma_start(bq_row, bass.AP(tensor=bq.tensor, offset=bq.offset,
                                          ap=[[0, 1], list(bq.ap[0])]))
        wq_st = stage.tile([P, 2, EMB], F32, tag="wst", bufs=3, name="wq_st")
        nc.sync.dma_start(wq_st, Wq.rearrange("(t p) m -> p t m", p=P))
        wk_st = stage.tile([P, 2, EMB], F32, tag="wst", bufs=3, name="wk_st")
        nc.sync.dma_start(wk_st, Wk.rearrange("(t p) m -> p t m", p=P))
        dma_x_batch(1)
        wv_st = stage.tile([P, 2, EMB], F32, tag="wst", bufs=3, name="wv_st")
        nc.sync.dma_start(wv_st, Wv.rearrange("(t p) m -> p t m", p=P))
        dma_x_batch(2)
        dma_x_batch(3)
        bv_bc = consts.tile([P, EMB], F32)
        nc.sync.dma_start(
            bv_bc,
            bass.AP(tensor=bv.tensor, offset=bv.offset, ap=[[0, P], list(bv.ap[0])]),
        )
        for bi in range(4, len(X_BATCHES)):
            dma_x_batch(bi)

        # ---- weights: A = Wq^T @ Wk, u = bq @ Wk, WvT ----
        # Wq on DVE, Wk on Act: the casts run in parallel so A starts earliest
        wq_bf = persist.tile([P, 2, EMB], BF16)
        nc.vector.tensor_copy(wq_bf, wq_st)
        wk_bf = persist.tile([P, 2, EMB], BF16)
        nc.vector.tensor_copy(wk_bf, wk_st)
        wv_bf = persist.tile([P, 2, EMB], BF16)
        nc.scalar.copy(wv_bf, wv_st)
        bq_bf = consts.tile([P, 2], BF16)
        for ec in range(2):
            btp = ps.tile([P, 1], F32, tag="sc", name=f"btp{ec}")
            nc.tensor.transpose(btp, bq_row[0:1, ec * P:(ec + 1) * P],
                                ones_f[0:1, 0:1])
            nc.vector.tensor_copy(bq_bf[:, ec:ec + 1], btp)

        A_sb = persist.tile([P, 2, EMB], BF16)
        WvT = persist.tile([P, 2, EMB], BF16)
        u_col = consts.tile([P, 2], F32)

        def emit_weights():
            for dc in range(2):
                aps = ps.tile([P, EMB], F32, tag="sc", name=f"aps{dc}")
                for ec in range(2):
                    nc.tensor.matmul(aps, wq_bf[:, ec, dc * P:(dc + 1) * P],
                                     wk_bf[:, ec, :],
                                     start=(ec == 0), stop=(ec == 1))
                nc.vector.tensor_copy(A_sb[:, dc, :], aps)
            u_ps = ps.tile([1, EMB], F32, tag="sc")
            for ec in range(2):
                nc.tensor.matmul(u_ps, bq_bf[:, ec:ec + 1], wk_bf[:, ec, :],
                                 start=(ec == 0), stop=(ec == 1))
            u_sb = work.tile([1, EMB], F32, tag="u_sb")
            nc.vector.tensor_copy(u_sb, u_ps)
            for jc in range(2):
                utp = ps.tile([P, 1], F32, tag="sc", name=f"utp{jc}")
                nc.tensor.transpose(utp, u_sb[0:1, jc * P:(jc + 1) * P],
                                    ones_f[0:1, 0:1])
                nc.vector.tensor_copy(u_col[:, jc:jc + 1], utp)
            for dc in range(2):
                for et in range(2):
                    tp = ps.tile([P, P], BF16, tag="sc", name=f"wvtp{dc}{et}")
                    nc.tensor.transpose(tp, wv_bf[:, et, dc * P:(dc + 1) * P],
                                        idb)
                    nc.vector.tensor_copy(WvT[:, dc, et * P:(et + 1) * P], tp)

        # ---- x: cast, PE-transpose -> xT[d, s]; project V and Q' ----
        # 4-tile groups share one PSUM tile per stage (transposes, V, Q') so
        # the 2-slot PSUM rotation amortizes the cross-engine copy latency;
        # V(g) and Q'(g) trail the transposes of group g+1.
        xT = persist.tile([P, 2, s_len], BF16, name="xT")
        QpT = persist.tile([P, 2, s_len], BF16, name="QpT")
        Vb = persist.tile([P, n_st, EMB], BF16, name="Vb")
        GT = 4   # tiles per group

        def emit_vqp(g):
            gsl = slice(g * GT * P, (g + 1) * GT * P)
            # the PV accumulator bank-group is idle during the front: use
            # it for the V-projection batches so the "sc" rotation only has
            # to cycle the transpose and Q' tiles
            vB = ps.tile([P, GT, EMB], F32, tag="po", bufs=1, name=f"vB{g}")
            for tl in range(GT):
                tsl = slice((g * GT + tl) * P, (g * GT + tl + 1) * P)
                for dc in range(2):
                    nc.tensor.matmul(vB[:, tl, :], xT[:, dc, tsl], WvT[:, dc, :],
                                     start=(dc == 0), stop=(dc == 1))
            nc.vector.tensor_copy(Vb[:, g * GT:(g + 1) * GT, :], vB)
            qpB = ps.tile([P, 2, HB], F32, tag="sc", name=f"qpB{g}")
            for jc in range(2):
                for dc in range(2):
                    nc.tensor.matmul(qpB[:, jc, :], A_sb[:, dc, jc * P:(jc + 1) * P],
                                     xT[:, dc, gsl],
                                     start=(dc == 0), stop=(dc == 1))
            for jc in range(2):
                nc.scalar.activation(QpT[:, jc, gsl], qpB[:, jc, :], AF.Identity,
                                     bias=u_col[:, jc:jc + 1], scale=1.0)

        g_i = 0
        for bi, nb in enumerate(X_BATCHES):
            xbf = stage.tile([P, nb, EMB], BF16, tag="xbf", name=f"xbf{bi}")
            nc.gpsimd.tensor_copy(xbf, xst[bi])
            for t0g in range(0, nb, GT):
                gsl = slice(g_i * GT * P, (g_i + 1) * GT * P)
                tpB = ps.tile([P, 2, GT * P], BF16, tag="sc", name=f"tpB{g_i}")
                for tl in range(GT):
                    for dc in range(2):
                        nc.tensor.transpose(
                            tpB[:, dc, tl * P:(tl + 1) * P],
                            xbf[:, t0g + tl, dc * P:(dc + 1) * P], idb)
                nc.vector.tensor_copy(xT[:, :, gsl], tpB)
                if g_i == 0:
                    emit_weights()   # fills the PE while x batch 1 lands
                if g_i >= 2:
                    emit_vqp(g_i - 2)
                g_i += 1
        emit_vqp(g_i - 2)
        emit_vqp(g_i - 1)

        # ---- attention ----
        # q-blocks of (start, n_half) in 512-wide halves; the narrower final
        # blocks shorten the end-of-kernel drain (PV lag + finalize chain).
        # Per k-tile: ONE [128, n_h*512] PSUM score tile (bank per half), ONE
        # exp, ONE DVE denominator accumulate - minimizes the per-instruction
        # semaphore-wait overhead on the PE stream.
        qblocks = [(0, 2), (1024, 2), (2048, 2), (3072, 1), (3584, 1)]
        for qb_i, (q0b, n_h) in enumerate(qblocks):
            nq = n_h * 4   # 128-row q-tiles in this block
            # deep lag mid-kernel so the next block's PV start always lands
            # after this block's finalize; shallow on the last block so the
            # end-of-kernel drain is short
            LAG = 2 if qb_i == len(qblocks) - 1 else 4
            out_ps = ps.tile([P, 8, EMB], F32, tag="po", bufs=1,
                             name=f"out_ps_{qb_i}")
            # two interleaved denominator accumulators (DVE + gpsimd) so
            # neither chain lags the PE and holds exp buffers alive
            dacc = [work.tile([P, n_h, HB], F32, tag=f"dacc{i}", bufs=2,
                              name=f"dacc{i}_{qb_i}") for i in range(2)]
            elist = []

            def emit_pv(kp):
                for h in range(n_h):
                    for j in range(4):
                        jg = h * 4 + j
                        nc.tensor.matmul(out_ps[:, jg, :],
                                         elist[kp][:, h, j * P:(j + 1) * P],
                                         Vb[:, kp, :],
                                         start=(kp == 0 and jg % 2 == 0),
                                         stop=(kp == n_kt - 1 and jg % 2 == 1))

            for kt_i in range(n_kt):
                ksl = slice(kt_i * P, (kt_i + 1) * P)
                sc = ps.tile([P, n_h, HB], F32, tag="sc",
                             name=f"sc{qb_i}_{kt_i}")
                for dc in range(2):   # lhsT reused across halves: 1 LDWEIGHTS
                    for h in range(n_h):
                        hsl = slice(q0b + h * HB, q0b + (h + 1) * HB)
                        nc.tensor.matmul(sc[:, h, :], xT[:, dc, ksl],
                                         QpT[:, dc, hsl],
                                         start=(dc == 0), stop=(dc == 1))
                ebf = work.tile([P, n_h, HB], BF16, tag="E", bufs=10,
                                name=f"e{qb_i}_{kt_i}")
                nc.scalar.activation(ebf, sc, AF.Exp, scale=scale)
                if kt_i < n_kt - 1:   # last tile's sum comes straight from ebf
                    ci = kt_i % 2
                    eng = nc.vector if ci == 0 else nc.gpsimd
                    da = dacc[ci]
                    if kt_i < 2:
                        eng.tensor_copy(da, ebf)
                    else:
                        eng.tensor_add(da, da, ebf)
                elist.append(ebf)
                if kt_i >= LAG:
                    emit_pv(kt_i - LAG)
            # denominators: tiny N=1 matmuls chunk.T @ ones -> [q, 1] columns
            # in one PSUM bank (an "sc" slot, free during the boundary). The
            # last k-tile's term reads the exp tile directly so the chain
            # tails don't gate the finalize; emitting them one catchup k-tile
            # in lets exp(last) land first, and the early start lets the stt
            # chain finish before the next block's first PV needs out_ps.
            dn_ps = ps.tile([P, nq], F32, tag="sc", name=f"dn_{qb_i}")

            def emit_dn():
                srcs = [(dacc[0], ones_f), (dacc[1], ones_f),
                        (elist[-1], ones_bf)]
                for si, (dsrc, drhs) in enumerate(srcs):
                    for j in range(nq):
                        nc.tensor.matmul(
                            dn_ps[:, j:j + 1],
                            dsrc[:, j // 4, (j % 4) * P:(j % 4 + 1) * P], drhs,
                            start=(si == 0 and j == 0),
                            stop=(si == 2 and j == nq - 1))

            for ci, kp in enumerate(range(n_kt - LAG, n_kt)):
                emit_pv(kp)
                if ci == 0:
                    emit_dn()
            recip = work.tile([P, 8], F32, tag="recip", name=f"recip{qb_i}")
            nc.vector.reciprocal(recip[:, 0:nq], dn_ps)
            ost = work.tile([P, 8, EMB], F32, tag="ost", name=f"ost{qb_i}")
            for j in range(nq):
                nc.vector.scalar_tensor_tensor(
                    ost[:, j, :], out_ps[:, j, :], recip[:, j:j + 1], bv_bc,
                    op0=mybir.AluOpType.mult, op1=mybir.AluOpType.add)
            last = qb_i == len(qblocks) - 1
            chunk = 1 if last else 2
            for ci in range(nq // chunk):
                q0 = q0b + ci * chunk * P
                dst = bass.AP(
                    tensor=out.tensor, offset=out.offset + q0 * EMB,
                    ap=[[EMB, P], [P * EMB, chunk], [1, EMB]])
                nc.sync.dma_start(dst, ost[:, ci * chunk:(ci + 1) * chunk, :])


def _make_nc(s_len: int = S) -> bass.Bass:
    # Bacc (not raw Bass): its compile() splits multi-sem waits and moves
    # matmul waits onto ldweights - HW allows at most one wait per inst.
    nc = bacc.Bacc("TRN2", target_bir_lowering=False, debug=False)
    _build(nc, s_len)
    nc.compile()
    return nc


def _prep(inputs: dict) -> dict:
    arrs = {k: np.ascontiguousarray(np.asarray(v, dtype=np.float32))
            for k, v in inputs.items()}
    assert arrs["x"].shape == (B, S, EMB), arrs["x"].shape
    return arrs


def run(inputs: dict):
    """Run on 8 NeuronCores. Returns (out[B,S,E] f32, BassKernelResults)."""
    arrs = _prep(inputs)
    nc = _make_nc(S)
    shared = {k: arrs[k] for k in ("Wq", "bq", "Wk", "Wv", "bv")}
    in_maps = [dict(shared, x=arrs["x"][i]) for i in range(B)]
    res = bass_utils.run_bass_kernel_spmd(nc, in_maps, core_ids=list(range(B)))
    out = np.stack([r["out"] for r in res.results], axis=0).astype(np.float32)
    return out, res


def kernel(**inputs) -> np.ndarray:
    out, _ = run(inputs)
    return out


def bench(inputs: dict, iters: int = 5, chain: int = 1):
    """Compile once, then time repeated executions with device-resident
    inputs (mirrors bass2jax.run_bass_via_pjrt's multi-core path).

    `chain` > 1 executes the NEFF that many times inside one XLA program
    (each call's outputs feed the next call's donated output buffers, which
    serializes them) so per-iteration device time can be extracted as a
    slope, amortizing the axon dispatch overhead.

    Returns (out[B,S,E] f32, list of per-call wall times in seconds).
    """
    import time

    import jax
    from jax.sharding import Mesh, NamedSharding, PartitionSpec
    from jax.experimental.shard_map import shard_map

    from concourse import bass2jax
    from concourse import mybir as mb

    arrs = _prep(inputs)
    nc = _make_nc(S)
    bass2jax.install_neuronx_cc_hook()

    partition_name = (
        nc.partition_id_tensor.name if nc.partition_id_tensor else None
    )
    in_names, out_names, out_avals, zero_outs = [], [], [], []
    for alloc in nc.m.functions[0].allocations:
        if not isinstance(alloc, mb.MemoryLocationSet):
            continue
        name = alloc.memorylocations[0].name
        if alloc.kind == "ExternalInput":
            if name != partition_name:
                in_names.append(name)
        elif alloc.kind == "ExternalOutput":
            out_names.append(name)
            shape = tuple(alloc.tensor_shape)
            dtype = mb.dt.np(alloc.dtype)
            out_avals.append(jax.core.ShapedArray(shape, dtype))
            zero_outs.append(np.zeros(shape, dtype))
    n_params = len(in_names)
    n_outs = len(out_avals)
    all_names = in_names + out_names
    if partition_name is not None:
        all_names = all_names + [partition_name]

    def _call(ins, zeros):
        operands = list(ins) + list(zeros)
        if partition_name is not None:
            operands.append(bass2jax.partition_id_tensor())
        return bass2jax._bass_exec_p.bind(
            *operands,
            out_avals=tuple(out_avals),
            in_names=tuple(all_names),
            out_names=tuple(out_names),
            lowering_input_output_aliases=(),
            sim_require_finite=True,
            sim_require_nnan=True,
            nc=nc,
        )

    def _body(*args):
        ins = list(args[:n_params])
        zeros = list(args[n_params:])
        outs = _call(ins, zeros)
        for _ in range(chain - 1):
            outs = _call(ins, list(outs))
        return tuple(outs)

    devices = jax.devices()[:B]
    mesh = Mesh(np.asarray(devices), ("core",))
    in_specs = (PartitionSpec("core"),) * (n_params + n_outs)
    out_specs = (PartitionSpec("core"),) * n_outs
    donate = tuple(range(n_params, n_params + n_outs))
    sharded = jax.jit(
        shard_map(_body, mesh=mesh, in_specs=in_specs, out_specs=out_specs,
                  check_rep=False),
        donate_argnums=donate,
        keep_unused=True,
    )

    per_core = [
        [arrs["x"][c] if n == "x" else arrs[n] for n in in_names[:n_params]]
        for c in range(B)
    ]
    concat_in = [
        np.concatenate([per_core[c][i] for c in range(B)], axis=0)
        for i in range(n_params)
    ]
    concat_zeros = [
        np.zeros((B * z.shape[0], *z.shape[1:]), z.dtype) for z in zero_outs
    ]

    shard = NamedSharding(mesh, PartitionSpec("core"))
    dev_in = [jax.device_put(a, shard) for a in concat_in]
    jax.block_until_ready(dev_in)

    times = []
    out_np = None
    for i in range(iters + 1):
        dev_zeros = [jax.device_put(z, shard) for z in concat_zeros]
        jax.block_until_ready(dev_zeros)
        t0 = time.perf_counter()
        outs = sharded(*dev_in, *dev_zeros)
        jax.block_until_ready(outs)
        dt = time.perf_counter() - t0
        if i == 0:
            idx = out_names.index("out")
            out_np = np.asarray(outs[idx]).reshape(B, S, EMB).astype(np.float32)
        else:
            times.append(dt)
    return out_np, times


# revision 32
# speedup vs baseline: 1.5070x; 1.3080x over previous
"""Single-head MHA (QKV proj + softmax attention) on 8 Trainium2 cores.

Problem: x[8, 4096, 256] f32; per-batch attention with per-head emb 256.
Sharding: data-parallel - one batch element per NeuronCore (8 cores).

Per-core algorithm (S=4096, E=256, P=128 partitions), all matmuls bf16:
  - A = Wq^T @ Wk [256, 256] once (tiny), so scores = (x @ A) @ x^T and the
    K projection disappears; the bq bias folds in exactly as a per-partition
    column u = bq @ Wk on the Q' projection, and the bk bias term is
    constant per q-row so it cancels in softmax.
  - x arrives in 5 batched DMAs; per 128-row tile: cast to bf16 (gpsimd),
    PE-transpose into xT[d, s], V-tile = xT.T @ WvT, and per 512 columns
    Q'T[e', s] = A.T @ xT (+u bias fused in the PSUM->SBUF copy).
  - attention per q-block of 1024 columns, two 512-wide halves per k-tile:
      S^T[k, qh] = xT_slice.T @ Q'T   (2 matmuls, fp32 PSUM, 1-bank tiles)
      E[k, qh]   = exp(S^T / 16)      (ScalarE, scale fused, bf16 out)
      out[q, e] += E_chunk.T @ V      (4 matmuls N=256 per half, lagged 4
                                       k-tiles (2 on the last block); E
                                       q-chunks stationary so the output
                                       lands in [q, e] - no transposes)
      denominators: two interleaved DVE/gpsimd accumulator chains sum the
      exp tiles; at the block boundary tiny N=1 matmuls (chunk.T @ ones)
      reduce them across partitions into one PSUM bank, with the last
      k-tile's term read straight from its exp tile so the chain tails
      never gate the finalize.
    finalize: recip (DVE), then out = out_ps*recip + bv per 128-row tile
    (softmax rows sum to 1, so attn @ (V + bv) = attn @ V + bv) on DVE,
    written to a staging tile and DMA'd out in 256-row chunks. No PE
    instruction depends on the finalize, so the PE streams straight into
    the next q-block.

PSUM: [128,2,512]f32 score tiles (2 slots, bank per half) + [128,8,256]f32
PV accumulator (4 banks, doubles as the front-end V-projection slot) + a
boundary-time denominator bank borrowed from the score rotation.

No running-max subtraction: scores/16 ~ N(0,1); max observed ~10.5, exp
stays well inside fp32/bf16 range.
"""

from contextlib import ExitStack

import numpy as np

import concourse.bass as bass
import concourse.tile as tile
from concourse import bacc
from concourse import mybir
from concourse import bass_utils
from concourse.masks import make_identity

P = 128          # partitions
EMB = 256        # head dim
S = 4096         # sequence length
B = 8            # batch == number of cores
QB = 1024        # q-block
HB = 512         # q-half (one PSUM bank of fp32)

F32 = mybir.dt.float32
BF16 = mybir.dt.bfloat16
FP8 = mybir.dt.float8e4
AF = mybir.ActivationFunctionType

X_BATCHES = (4,) * 8   # 128-row x tiles per input DMA


def _build(nc: bass.Bass, s_len: int = S) -> None:
    """Emit the per-core program into `nc` (SPMD: same program all cores)."""
    x = nc.dram_tensor("x", (s_len, EMB), F32, kind="ExternalInput").ap()
    Wq = nc.dram_tensor("Wq", (EMB, EMB), F32, kind="ExternalInput").ap()
    bq = nc.dram_tensor("bq", (EMB,), F32, kind="ExternalInput").ap()
    Wk = nc.dram_tensor("Wk", (EMB, EMB), F32, kind="ExternalInput").ap()
    Wv = nc.dram_tensor("Wv", (EMB, EMB), F32, kind="ExternalInput").ap()
    bv = nc.dram_tensor("bv", (EMB,), F32, kind="ExternalInput").ap()
    out = nc.dram_tensor("out", (s_len, EMB), F32, kind="ExternalOutput").ap()

    n_st = s_len // P      # 128-row tiles of the sequence
    n_qb = s_len // QB     # q-blocks
    n_kt = s_len // P      # k-tiles
    n_qt = QB // P         # 128-row q-tiles per q-block
    scale = float(EMB) ** -0.5

    with tile.TileContext(nc) as tc, ExitStack() as ctx:
        consts = ctx.enter_context(tc.tile_pool(name="consts", bufs=1))
        persist = ctx.enter_context(tc.tile_pool(name="persist", bufs=1))
        stage = ctx.enter_context(tc.tile_pool(name="stage", bufs=2))
        work = ctx.enter_context(tc.tile_pool(name="work", bufs=2))
        ps = ctx.enter_context(tc.tile_pool(name="ps", bufs=2, space="PSUM"))

        # ---- constants (no DMA deps: ready before the first transpose) ----
        idf = consts.tile([P, P], F32)
        make_identity(nc, idf)
        idb = consts.tile([P, P], BF16)
        nc.vector.tensor_copy(idb, idf)
        ones_f = consts.tile([P, 1], F32)
        nc.vector.memset(ones_f, 1.0)
        ones_bf = consts.tile([P, 1], BF16)
        nc.vector.memset(ones_bf, 1.0)
        eshift = consts.tile([P, 1], F32)
        nc.vector.memset(eshift, -5.5)

        # ---- input DMAs: everything else hides under them ----
        # order matters: HWDGE desc-gen and the DMA engines serialize; the
        # first PE work is x0 transposes, then A = f(Wq, Wk).
        xst = []
        t0 = 0

        def dma_x_batch(bi):
            nonlocal t0
            nb = X_BATCHES[bi]
            xb = stage.tile([P, nb, EMB], F32, tag="xst", name=f"xst{bi}")
            src = bass.AP(
                tensor=x.tensor, offset=x.offset + t0 * P * EMB,
                ap=[[EMB, P], [P * EMB, nb], [1, EMB]])
            nc.sync.dma_start(xb, src)
            xst.append(xb)
            t0 += nb

        dma_x_batch(0)
        bq_row = consts.tile([1, EMB], F32)
        nc.sync.dma_start(bq_row, bass.AP(tensor=bq.tensor, offset=bq.offset,
                                          ap=[[0, 1], list(bq.ap[0])]))
        wq_st = stage.tile([P, 2, EMB], F32, tag="wst", bufs=3, name="wq_st")
        nc.sync.dma_start(wq_st, Wq.rearrange("(t p) m -> p t m", p=P))
        wk_st = stage.tile([P, 2, EMB], F32, tag="wst", bufs=3, name="wk_st")
        nc.sync.dma_start(wk_st, Wk.rearrange("(t p) m -> p t m", p=P))
        dma_x_batch(1)
        wv_st = stage.tile([P, 2, EMB], F32, tag="wst", bufs=3, name="wv_st")
        nc.sync.dma_start(wv_st, Wv.rearrange("(t p) m -> p t m", p=P))
        dma_x_batch(2)
        dma_x_batch(3)
        bv_bc = consts.tile([P, EMB], F32)
        nc.sync.dma_start(
            bv_bc,
            bass.AP(tensor=bv.tensor, offset=bv.offset, ap=[[0, P], list(bv.ap[0])]),
        )
        for bi in range(4, len(X_BATCHES)):
            dma_x_batch(bi)

        # ---- weights: A = Wq^T @ Wk, u = bq @ Wk, WvT ----
        # Wq on DVE, Wk on Act: the casts run in parallel so A starts earliest
        wq_bf = persist.tile([P, 2, EMB], BF16)
        nc.vector.tensor_copy(wq_bf, wq_st)
        wk_bf = persist.tile([P, 2, EMB], BF16)
        nc.vector.tensor_copy(wk_bf, wk_st)
        wv_bf = persist.tile([P, 2, EMB], BF16)
        nc.scalar.copy(wv_bf, wv_st)
        bq_bf = consts.tile([P, 2], BF16)
        for ec in range(2):
            btp = ps.tile([P, 1], F32, tag="sc", name=f"btp{ec}")
            nc.tensor.transpose(btp, bq_row[0:1, ec * P:(ec + 1) * P],
                                ones_f[0:1, 0:1])
            nc.vector.tensor_copy(bq_bf[:, ec:ec + 1], btp)

        A_sb = persist.tile([P, 2, EMB], BF16)
        WvT = persist.tile([P, 2, EMB], BF16)
        u_col = consts.tile([P, 2], F32)

        def emit_weights():
            for dc in range(2):
                aps = ps.tile([P, EMB], F32, tag="sc", name=f"aps{dc}")
                for ec in range(2):
                    nc.tensor.matmul(aps, wq_bf[:, ec, dc * P:(dc + 1) * P],
                                     wk_bf[:, ec, :],
                                     start=(ec == 0), stop=(ec == 1))
                nc.vector.tensor_copy(A_sb[:, dc, :], aps)
            u_ps = ps.tile([1, EMB], F32, tag="sc")
            for ec in range(2):
                nc.tensor.matmul(u_ps, bq_bf[:, ec:ec + 1], wk_bf[:, ec, :],
                                 start=(ec == 0), stop=(ec == 1))
            u_sb = work.tile([1, EMB], F32, tag="u_sb")
            nc.vector.tensor_copy(u_sb, u_ps)
            for jc in range(2):
                utp = ps.tile([P, 1], F32, tag="sc", name=f"utp{jc}")
                nc.tensor.transpose(utp, u_sb[0:1, jc * P:(jc + 1) * P],
                                    ones_f[0:1, 0:1])
                nc.vector.tensor_copy(u_col[:, jc:jc + 1], utp)
            for dc in range(2):
                for et in range(2):
                    tp = ps.tile([P, P], BF16, tag="sc", name=f"wvtp{dc}{et}")
                    nc.tensor.transpose(tp, wv_bf[:, et, dc * P:(dc + 1) * P],
                                        idb)
                    nc.vector.tensor_copy(WvT[:, dc, et * P:(et + 1) * P], tp)

        # ---- x: cast, PE-transpose -> xT[d, s]; project V and Q' ----
        # 4-tile groups share one PSUM tile per stage (transposes, V, Q') so
        # the 2-slot PSUM rotation amortizes the cross-engine copy latency;
        # V(g) and Q'(g) trail the transposes of group g+1.
        xT = persist.tile([P, 2, s_len], BF16, name="xT")
        QpT = persist.tile([P, 2, s_len], BF16, name="QpT")
        Vb = persist.tile([P, n_st, EMB], FP8, name="Vb")
        GT = 4   # tiles per group

        def emit_vqp(g):
            gsl = slice(g * GT * P, (g + 1) * GT * P)
            # the PV accumulator bank-group is idle during the front: use
            # it for the V-projection batches so the "sc" rotation only has
            # to cycle the transpose and Q' tiles
            vB = ps.tile([P, GT, EMB], F32, tag="po", bufs=1, name=f"vB{g}")
            for tl in range(GT):
                tsl = slice((g * GT + tl) * P, (g * GT + tl + 1) * P)
                for dc in range(2):
                    nc.tensor.matmul(vB[:, tl, :], xT[:, dc, tsl], WvT[:, dc, :],
                                     start=(dc == 0), stop=(dc == 1))
            nc.vector.tensor_copy(Vb[:, g * GT:(g + 1) * GT, :], vB)
            qpB = ps.tile([P, 2, HB], F32, tag="sc", name=f"qpB{g}")
            for jc in range(2):
                for dc in range(2):
                    nc.tensor.matmul(qpB[:, jc, :], A_sb[:, dc, jc * P:(jc + 1) * P],
                                     xT[:, dc, gsl],
                                     start=(dc == 0), stop=(dc == 1))
            for jc in range(2):
                nc.scalar.activation(QpT[:, jc, gsl], qpB[:, jc, :], AF.Identity,
                                     bias=u_col[:, jc:jc + 1], scale=1.0)

        g_i = 0
        for bi, nb in enumerate(X_BATCHES):
            xbf = stage.tile([P, nb, EMB], BF16, tag="xbf", name=f"xbf{bi}")
            nc.gpsimd.tensor_copy(xbf, xst[bi])
            for t0g in range(0, nb, GT):
                gsl = slice(g_i * GT * P, (g_i + 1) * GT * P)
                tpB = ps.tile([P, 2, GT * P], BF16, tag="sc", name=f"tpB{g_i}")
                for tl in range(GT):
                    for dc in range(2):
                        nc.tensor.transpose(
                            tpB[:, dc, tl * P:(tl + 1) * P],
                            xbf[:, t0g + tl, dc * P:(dc + 1) * P], idb)
                nc.vector.tensor_copy(xT[:, :, gsl], tpB)
                if g_i == 0:
                    emit_weights()   # fills the PE while x batch 1 lands
                if g_i >= 2:
                    emit_vqp(g_i - 2)
                g_i += 1
        emit_vqp(g_i - 2)
        emit_vqp(g_i - 1)

        # ---- attention ----
        # q-blocks of (start, n_half) in 512-wide halves; the narrower final
        # blocks shorten the end-of-kernel drain (PV lag + finalize chain).
        # Per k-tile: ONE [128, n_h*512] PSUM score tile (bank per half), ONE
        # exp, ONE DVE denominator accumulate - minimizes the per-instruction
        # semaphore-wait overhead on the PE stream.
        qblocks = [(0, 2), (1024, 2), (2048, 2), (3072, 1), (3584, 1)]
        for qb_i, (q0b, n_h) in enumerate(qblocks):
            nq = n_h * 4   # 128-row q-tiles in this block
            # deep lag mid-kernel so the next block's PV start always lands
            # after this block's finalize; shallow on the last block so the
            # end-of-kernel drain is short
            LAG = 2 if qb_i == len(qblocks) - 1 else 4
            out_ps = ps.tile([P, 8, EMB], F32, tag="po", bufs=1,
                             name=f"out_ps_{qb_i}")
            # two interleaved denominator accumulators (DVE + gpsimd) so
            # neither chain lags the PE and holds exp buffers alive
            dacc = [work.tile([P, n_h, HB], F32, tag=f"dacc{i}", bufs=2,
                              name=f"dacc{i}_{qb_i}") for i in range(2)]
            elist = []
            n_pair = n_kt // 2
            PLAG = 1 if qb_i == len(qblocks) - 1 else 2

            def emit_pv(pc, elist=elist, out_ps=out_ps, n_h=n_h):
                # fp8 DoubleRow: one matmul contracts a 256-deep k-chunk
                # (the epair tile holds two k-tiles of exp output)
                for h in range(n_h):
                    for j in range(4):
                        jg = h * 4 + j
                        nc.tensor.matmul(
                            out_ps[:, jg, :],
                            elist[pc][:, :, h, j * P:(j + 1) * P],
                            Vb[:, 2 * pc:2 * pc + 2, :],
                            start=(pc == 0 and jg % 2 == 0),
                            stop=(pc == n_pair - 1 and jg % 2 == 1),
                            perf_mode=mybir.MatmulPerfMode.DoubleRow)

            for kt_i in range(n_kt):
                ksl = slice(kt_i * P, (kt_i + 1) * P)
                sc = ps.tile([P, n_h, HB], F32, tag="sc",
                             name=f"sc{qb_i}_{kt_i}")
                for dc in range(2):   # lhsT reused across halves: 1 LDWEIGHTS
                    for h in range(n_h):
                        hsl = slice(q0b + h * HB, q0b + (h + 1) * HB)
                        nc.tensor.matmul(sc[:, h, :], xT[:, dc, ksl],
                                         QpT[:, dc, hsl],
                                         start=(dc == 0), stop=(dc == 1))
                if kt_i % 2 == 0:
                    epair = work.tile([P, 2, n_h, HB], FP8, tag="E", bufs=6,
                                      name=f"e{qb_i}_{kt_i // 2}")
                    elist.append(epair)
                # constant shift keeps exp inside fp8 range (max score/16
                # ~10.5 -> e^5 = 148 < 240); softmax divides it back out
                nc.scalar.activation(epair[:, kt_i % 2, :, :], sc, AF.Exp,
                                     bias=eshift, scale=scale)
                if kt_i < n_kt - 1:  # last tile's sum comes straight from ebf
                    ci = kt_i % 2
                    eng = nc.vector if ci == 0 else nc.gpsimd
                    da = dacc[ci]
                    if kt_i < 2:
                        eng.tensor_copy(da, epair[:, ci, :, :])
                    else:
                        eng.tensor_add(da, da, epair[:, ci, :, :])
                if kt_i % 2 == 1 and kt_i // 2 >= PLAG:
                    emit_pv(kt_i // 2 - PLAG)

            # denominators: tiny N=1 matmuls chunk.T @ ones -> [q, 1]
            # columns in one PSUM bank (an "sc" slot, free during the
            # boundary). The last k-tile's term reads the exp tile directly
            # so the chain tails don't gate the finalize.
            dn_ps = ps.tile([P, nq], F32, tag="sc", name=f"dn_{qb_i}")

            def emit_dn():
                srcs = [(dacc[0], ones_f), (dacc[1], ones_f),
                        (elist[-1][:, 1, :, :], ones_bf)]
                for si, (dsrc, drhs) in enumerate(srcs):
                    for j in range(nq):
                        nc.tensor.matmul(
                            dn_ps[:, j:j + 1],
                            dsrc[:, j // 4, (j % 4) * P:(j % 4 + 1) * P], drhs,
                            start=(si == 0 and j == 0),
                            stop=(si == 2 and j == nq - 1))

            for pc in range(n_pair - PLAG, n_pair):
                emit_pv(pc)
            emit_dn()
            recip = work.tile([P, 8], F32, tag="recip", name=f"recip{qb_i}")
            nc.vector.reciprocal(recip[:, 0:nq], dn_ps)
            ost = work.tile([P, 8, EMB], F32, tag="ost", name=f"ost{qb_i}")
            for j in range(nq):
                nc.vector.scalar_tensor_tensor(
                    ost[:, j, :], out_ps[:, j, :], recip[:, j:j + 1], bv_bc,
                    op0=mybir.AluOpType.mult, op1=mybir.AluOpType.add)
            last = qb_i == len(qblocks) - 1
            chunk = 1 if last else 2
            for ci in range(nq // chunk):
                q0 = q0b + ci * chunk * P
                dst = bass.AP(
                    tensor=out.tensor, offset=out.offset + q0 * EMB,
                    ap=[[EMB, P], [P * EMB, chunk], [1, EMB]])
                nc.sync.dma_start(dst, ost[:, ci * chunk:(ci + 1) * chunk, :])


def _make_nc(s_len: int = S) -> bass.Bass:
    # Bacc (not raw Bass): its compile() splits multi-sem waits and moves
    # matmul waits onto ldweights - HW allows at most one wait per inst.
    nc = bacc.Bacc("TRN2", target_bir_lowering=False, debug=False)
    _build(nc, s_len)
    nc.compile()
    return nc


def _prep(inputs: dict) -> dict:
    arrs = {k: np.ascontiguousarray(np.asarray(v, dtype=np.float32))
            for k, v in inputs.items()}
    assert arrs["x"].shape == (B, S, EMB), arrs["x"].shape
    return arrs


def run(inputs: dict):
    """Run on 8 NeuronCores. Returns (out[B,S,E] f32, BassKernelResults)."""
    arrs = _prep(inputs)
    nc = _make_nc(S)
    shared = {k: arrs[k] for k in ("Wq", "bq", "Wk", "Wv", "bv")}
    in_maps = [dict(shared, x=arrs["x"][i]) for i in range(B)]
    res = bass_utils.run_bass_kernel_spmd(nc, in_maps, core_ids=list(range(B)))
    out = np.stack([r["out"] for r in res.results], axis=0).astype(np.float32)
    return out, res


def kernel(**inputs) -> np.ndarray:
    out, _ = run(inputs)
    return out


def bench(inputs: dict, iters: int = 5, chain: int = 1):
    """Compile once, then time repeated executions with device-resident
    inputs (mirrors bass2jax.run_bass_via_pjrt's multi-core path).

    `chain` > 1 executes the NEFF that many times inside one XLA program
    (each call's outputs feed the next call's donated output buffers, which
    serializes them) so per-iteration device time can be extracted as a
    slope, amortizing the axon dispatch overhead.

    Returns (out[B,S,E] f32, list of per-call wall times in seconds).
    """
    import time

    import jax
    from jax.sharding import Mesh, NamedSharding, PartitionSpec
    from jax.experimental.shard_map import shard_map

    from concourse import bass2jax
    from concourse import mybir as mb

    arrs = _prep(inputs)
    nc = _make_nc(S)
    bass2jax.install_neuronx_cc_hook()

    partition_name = (
        nc.partition_id_tensor.name if nc.partition_id_tensor else None
    )
    in_names, out_names, out_avals, zero_outs = [], [], [], []
    for alloc in nc.m.functions[0].allocations:
        if not isinstance(alloc, mb.MemoryLocationSet):
            continue
        name = alloc.memorylocations[0].name
        if alloc.kind == "ExternalInput":
            if name != partition_name:
                in_names.append(name)
        elif alloc.kind == "ExternalOutput":
            out_names.append(name)
            shape = tuple(alloc.tensor_shape)
            dtype = mb.dt.np(alloc.dtype)
            out_avals.append(jax.core.ShapedArray(shape, dtype))
            zero_outs.append(np.zeros(shape, dtype))
    n_params = len(in_names)
    n_outs = len(out_avals)
    all_names = in_names + out_names
    if partition_name is not None:
        all_names = all_names + [partition_name]

    def _call(ins, zeros):
        operands = list(ins) + list(zeros)
        if partition_name is not None:
            operands.append(bass2jax.partition_id_tensor())
        return bass2jax._bass_exec_p.bind(
            *operands,
            out_avals=tuple(out_avals),
            in_names=tuple(all_names),
            out_names=tuple(out_names),
            lowering_input_output_aliases=(),
            sim_require_finite=True,
            sim_require_nnan=True,
            nc=nc,
        )

    def _body(*args):
        ins = list(args[:n_params])
        zeros = list(args[n_params:])
        outs = _call(ins, zeros)
        for _ in range(chain - 1):
            outs = _call(ins, list(outs))
        return tuple(outs)

    devices = jax.devices()[:B]
    mesh = Mesh(np.asarray(devices), ("core",))
    in_specs = (PartitionSpec("core"),) * (n_params + n_outs)
    out_specs = (PartitionSpec("core"),) * n_outs
    donate = tuple(range(n_params, n_params + n_outs))
    sharded = jax.jit(
        shard_map(_body, mesh=mesh, in_specs=in_specs, out_specs=out_specs,
                  check_rep=False),
        donate_argnums=donate,
        keep_unused=True,
    )

    per_core = [
        [arrs["x"][c] if n == "x" else arrs[n] for n in in_names[:n_params]]
        for c in range(B)
    ]
    concat_in = [
        np.concatenate([per_core[c][i] for c in range(B)], axis=0)
        for i in range(n_params)
    ]
    concat_zeros = [
        np.zeros((B * z.shape[0], *z.shape[1:]), z.dtype) for z in zero_outs
    ]

    shard = NamedSharding(mesh, PartitionSpec("core"))
    dev_in = [jax.device_put(a, shard) for a in concat_in]
    jax.block_until_ready(dev_in)

    times = []
    out_np = None
    for i in range(iters + 1):
        dev_zeros = [jax.device_put(z, shard) for z in concat_zeros]
        jax.block_until_ready(dev_zeros)
        t0 = time.perf_counter()
        outs = sharded(*dev_in, *dev_zeros)
        jax.block_until_ready(outs)
        dt = time.perf_counter() - t0
        if i == 0:
            idx = out_names.index("out")
            out_np = np.asarray(outs[idx]).reshape(B, S, EMB).astype(np.float32)
        else:
            times.append(dt)
    return out_np, times
